# revision 2
# baseline (speedup 1.0000x reference)
"""Trainium2 Bass kernel for nn_Branch1_block (gnn_message_passing), v2.

Data-parallel over batch on 8 NeuronCores (4 batches/core). All matmul
operands in bf16 (PSUM accumulation fp32): halves DMA + SBUF vs fp32r at
the same PE rate. T2 Chebyshev matrix precomputed on host (no on-chip
recurrence). SE attention is folded into per-j1 scaled copies of the
theta1 block-diagonal (no per-z scaling on DVE). xg2 never leaves SBUF:
the temporal-conv tail is interleaved per n-chunk into gconv2, and
LayerNorm stats accumulate per-chunk via bn_stats/bn_aggr.
"""
import sys

import numpy as np

try:
    import concourse.bass as bass
except ImportError:  # pragma: no cover - grading env fallback
    for p in ("/root/.axon_site", "/root/.axon_site/_ro/trn_rl_repo",
              "/root/.axon_site/_ro/pypackages", "/opt/trn_rl_repo"):
        if p not in sys.path:
            sys.path.append(p)
    import concourse.bass as bass

from contextlib import ExitStack

import ml_dtypes
import concourse.mybir as mybir
import concourse.tile as tile
from concourse import bacc
from concourse.bass_utils import run_bass_kernel_spmd

B, T, F, O, N, K = 32, 12, 16, 32, 2048, 3
NCORES = 8
BC = B // NCORES          # 4 batches per core
BT = BC * T               # 48
R1 = BT * F               # 768 rows (bt,f)
R2 = BT * O               # 1536 rows (bt,o)
J1 = R1 // 128            # 6
J2 = R2 // 128            # 12
NT = N // 128             # 16
NCH = 512
NNC = N // NCH            # 4
NTL = NCH // 128          # 4 n-tiles per chunk

f32 = mybir.dt.float32
f32r = mybir.dt.float32r
bf16 = mybir.dt.bfloat16
AF = mybir.ActivationFunctionType
AX = mybir.AxisListType
ALU = mybir.AluOpType

# (jt, ji) pairs with nonzero temporal-conv block matrices
TC_PAIRS = {0: (0, 2), 1: (0, 1), 2: (1, 2)}
# per j1 (r1 tile): the one or two batches its rows touch
B01 = []
for _j in range(6):
    _bs = sorted({(8 * _j + _p // 16) // 12 for _p in range(128)})
    B01.append((_bs[0], _bs[-1]))

_compiled = None
_compiled_affine = None


DEBUG_TAPS = False


def _build(ln_affine):
    nc = bacc.Bacc("TRN2", target_bir_lowering=False, debug=False)

    xT_d = nc.dram_tensor("xT", [N, R1], bf16, kind="ExternalInput").ap()
    xb_d = nc.dram_tensor("xb", [R1, N], f32, kind="ExternalInput").ap()
    cheb_d = nc.dram_tensor("cheb12", [2, N, N], bf16, kind="ExternalInput").ap()
    th1_d = nc.dram_tensor("th1", [128, 3 * 256], f32, kind="ExternalInput").ap()
    th2_d = nc.dram_tensor("th2", [128, 3 * 128], f32, kind="ExternalInput").ap()
    tcbd_d = nc.dram_tensor("tcbd", [128, 18 * 128], f32, kind="ExternalInput").ap()
    resbd_d = nc.dram_tensor("resbd", [128, 2 * 128], f32, kind="ExternalInput").ap()
    identb_d = nc.dram_tensor("identb", [128, 128], bf16, kind="ExternalInput").ap()
    selE1_d = nc.dram_tensor("selE1", [12, 12 * 128], bf16, kind="ExternalInput").ap()
    w1aug_d = nc.dram_tensor("w1aug", [13, 3], bf16, kind="ExternalInput").ap()
    w2aug_d = nc.dram_tensor("w2aug", [4, 12], bf16, kind="ExternalInput").ap()
    bias3_d = nc.dram_tensor("bias3", [128, 4], f32, kind="ExternalInput").ap()
    if ln_affine:
        lng_d = nc.dram_tensor("lng", [128, N], f32, kind="ExternalInput").ap()
        lnb_d = nc.dram_tensor("lnb", [128, N], f32, kind="ExternalInput").ap()
    y_d = nc.dram_tensor("y", [R2, N], f32, kind="ExternalOutput").ap()
    ysc_d = nc.dram_tensor("ysc", [R2, N], f32).ap()
    if DEBUG_TAPS:
        dbg_att_d = nc.dram_tensor("dbg_att", [128, J1], f32,
                                   kind="ExternalOutput").ap()
        dbg_xg1_d = nc.dram_tensor("dbg_xg1", [128, NT * R2], bf16,
                                   kind="ExternalOutput").ap()
        dbg_xg2_d = nc.dram_tensor("dbg_xg2", [R2, N], f32,
                                   kind="ExternalOutput").ap()

    dma = nc.sync.dma_start

    with tile.TileContext(nc) as tc, ExitStack() as top:
        cpool = top.enter_context(tc.tile_pool(name="const", bufs=1))
        th2_sb = cpool.tile([128, 3 * 128], f32r)
        tcbd_sb = cpool.tile([128, 18 * 128], f32r)
        resbd_sb = cpool.tile([128, 2 * 128], f32r)
        identb_sb = cpool.tile([128, 128], bf16)
        bias3_sb = cpool.tile([128, 4], f32)
        attc1 = cpool.tile([128, J1], f32)
        dma(th2_sb[:], th2_d.bitcast(f32r))
        dma(tcbd_sb[:], tcbd_d.bitcast(f32r))
        dma(resbd_sb[:], resbd_d.bitcast(f32r))
        dma(identb_sb[:], identb_d)
        dma(bias3_sb[:], bias3_d)
        if ln_affine:
            lng_sb = cpool.tile([128, N], f32)
            lnb_sb = cpool.tile([128, N], f32)
            dma(lng_sb[:], lng_d)
            dma(lnb_sb[:], lnb_d)

        xg1Tp = tc.tile_pool(name="xg1T", bufs=1)
        xg1Tpool = xg1Tp.__enter__()
        xg1T_sb = xg1Tpool.tile([128, NT, R2], bf16)

        chp = tc.tile_pool(name="chp", bufs=2)
        chpool = chp.__enter__()

        def load_ch(nci, name):
            ncs = nci * NCH
            ch = chpool.tile([128, 2, NT, NCH], bf16, tag="ch", bufs=2, name=name)
            chv = cheb_d[:, :, ncs:ncs + NCH].rearrange(
                "t (mi p) n -> t mi p n", p=128)
            for t_ in range(2):
                for q in range(2):
                    dma(ch[:, t_, q * 8:(q + 1) * 8, :],
                        chv[t_, q * 8:(q + 1) * 8].rearrange("mi p n -> p mi n"))
            return ch

        # ---------- phase A+B: attention + gconv1 ----------
        with tc.tile_pool(name="xTp", bufs=1) as xTpool, \
             tc.tile_pool(name="g1sb", bufs=2) as g1pool, \
             tc.tile_pool(name="attps", bufs=2, space="PSUM") as apsum, \
             tc.tile_pool(name="attsb", bufs=2) as aspool, \
             tc.tile_pool(name="zps", bufs=2, space="PSUM") as zpsum, \
             tc.tile_pool(name="fps", bufs=2, space="PSUM") as fpsum:
            xT_sb = xTpool.tile([128, NT, R1], bf16)
            th1_sb = xTpool.tile([128, 3 * 256], f32)
            th1s_sb = xTpool.tile([128, J1, 3 * 256], f32r)
            dma(th1_sb[:], th1_d)
            xTv = xT_d.rearrange("(mi p) r -> mi p r", p=128)
            for q in range(4):
                dma(xT_sb[:, q * 4:(q + 1) * 4, :],
                    xTv[q * 4:(q + 1) * 4].rearrange("mi p r -> p mi r"))
            ch0 = load_ch(0, "ch_0")
            ch1 = load_ch(1, "ch_1")

            # ---- SE attention ----
            ones_col = aspool.tile([128, 1], bf16, tag="ones")
            nc.vector.memset(ones_col[:], 1.0)
            rs48 = aspool.tile([1, BT], f32, tag="rs48")
            ident1 = aspool.tile([1, 1], f32, tag="ident1")
            nc.vector.memset(ident1[:], 1.0)
            for h in range(2):
                attps = apsum.tile([1, R1 // 2], f32, tag="attp")
                for mi in range(NT):
                    nc.tensor.matmul(attps[:], ones_col[:],
                                     xT_sb[:, mi, h * 384:(h + 1) * 384],
                                     start=(mi == 0), stop=(mi == NT - 1))
                nc.vector.reduce_sum(rs48[:, h * 24:(h + 1) * 24],
                                     attps[:].rearrange("p (a b) -> p a b", b=F),
                                     axis=AX.X)
            t48ps = apsum.tile([BT, 1], f32, tag="attp")
            nc.tensor.transpose(t48ps[:], rs48[:], ident1[:])
            att0sb = aspool.tile([BT, 1], bf16, tag="att0")
            nc.scalar.activation(att0sb[:], t48ps[:], AF.Copy, scale=1.0 / (F * N))
            atbps = apsum.tile([12, 4], f32, tag="attp")
            for b in range(4):
                nc.tensor.matmul(atbps[:, b:b + 1],
                                 identb_sb[:48, b * 12:(b + 1) * 12],
                                 att0sb[:], start=True, stop=True)
            atb13 = aspool.tile([13, 4], bf16, tag="atb13")
            nc.vector.memset(atb13[:], 1.0)
            nc.scalar.activation(atb13[:12, :], atbps[:], AF.Copy)
            w1aug_sb = aspool.tile([13, 3], bf16, tag="w1aug")
            w2aug_sb = aspool.tile([4, 12], bf16, tag="w2aug")
            selE1_sb = aspool.tile([12, 12 * 128], bf16, tag="selE1")
            dma(w1aug_sb[:], w1aug_d)
            dma(w2aug_sb[:], w2aug_d)
            dma(selE1_sb[:], selE1_d)
            a1ps = apsum.tile([3, 4], f32, tag="attp")
            nc.tensor.matmul(a1ps[:], w1aug_sb[:], atb13[:], start=True, stop=True)
            a1sb = aspool.tile([4, 4], bf16, tag="a1")
            nc.vector.memset(a1sb[:], 1.0)
            nc.scalar.activation(a1sb[:3, :], a1ps[:], AF.Relu)
            attps2 = apsum.tile([12, 4], f32, tag="attp")
            nc.tensor.matmul(attps2[:], w2aug_sb[:], a1sb[:], start=True, stop=True)
            att_tb = aspool.tile([12, 4], bf16, tag="att_tb")
            nc.scalar.activation(att_tb[:], attps2[:], AF.Sigmoid)
            for j in range(J1):
                b0, b1 = B01[j]
                acps = apsum.tile([128, 1], f32, tag="attp")
                nc.tensor.matmul(acps[:], selE1_sb[:, (j * 2) * 128:(j * 2 + 1) * 128],
                                 att_tb[:, b0:b0 + 1], start=True, stop=False)
                nc.tensor.matmul(acps[:], selE1_sb[:, (j * 2 + 1) * 128:(j * 2 + 2) * 128],
                                 att_tb[:, b1:b1 + 1], start=False, stop=True)
                nc.scalar.activation(attc1[:, j:j + 1], acps[:], AF.Copy)
                nc.vector.tensor_scalar_mul(th1s_sb[:, j, :], th1_sb[:],
                                            attc1[:, j:j + 1])

            # ---- gconv1: graph + feature per n-chunk ----
            chq = [ch0, ch1]
            for nci in range(NNC):
                ncs = nci * NCH
                ch = chq[nci] if nci < 2 else load_ch(nci, f"ch_{nci}")
                z0s = []
                for j1 in range(J1):
                    z0t = g1pool.tile([128, NCH], f32r, tag="z0", bufs=7,
                                      name=f"z0_{nci}_{j1}")
                    dma(z0t[:], xb_d[j1 * 128:(j1 + 1) * 128, ncs:ncs + NCH].bitcast(f32r))
                    z0s.append(z0t)

                def g1_feat(j1, zs):
                    z0t, z1t, z2t = zs
                    for ntl in range(NTL):
                        fps = fpsum.tile([128, 256], f32, tag="fps")
                        nc.tensor.matmul(fps[:],
                                         z0t[:, ntl * 128:(ntl + 1) * 128],
                                         th1s_sb[:, j1, 0:256],
                                         start=True, stop=False)
                        nc.tensor.matmul(fps[:],
                                         z1t[:, ntl * 128:(ntl + 1) * 128],
                                         th1s_sb[:, j1, 256:512],
                                         start=False, stop=False)
                        nc.tensor.matmul(fps[:],
                                         z2t[:, ntl * 128:(ntl + 1) * 128],
                                         th1s_sb[:, j1, 512:768],
                                         start=False, stop=True)
                        nc.scalar.activation(
                            xg1T_sb[:, nci * NTL + ntl, j1 * 256:(j1 + 1) * 256],
                            fps[:], AF.Relu)

                prev = None
                for j1 in range(J1):
                    zcur = [z0s[j1]]
                    for k in (1, 2):
                        zps = zpsum.tile([128, NCH], f32)
                        for mi in range(NT):
                            nc.tensor.matmul(zps[:],
                                             xT_sb[:, mi, j1 * 128:(j1 + 1) * 128],
                                             ch[:, k - 1, mi, :],
                                             start=(mi == 0), stop=(mi == NT - 1))
                        zt = g1pool.tile([128, NCH], f32r, tag=f"z{k}", bufs=3)
                        if k == 1:
                            nc.vector.tensor_copy(zt[:], zps[:])
                        else:
                            nc.scalar.activation(zt[:], zps[:], AF.Copy)
                        zcur.append(zt)
                    if prev is not None:
                        g1_feat(j1 - 1, prev)
                    prev = zcur
                g1_feat(J1 - 1, prev)

        if DEBUG_TAPS:
            dma(dbg_att_d, attc1[:])
            dma(dbg_xg1_d.rearrange("p (mi r) -> p mi r", r=R2), xg1T_sb[:])

        # ---------- phase C: gconv2 + temporal tail + LayerNorm ----------
        with tc.tile_pool(name="stat", bufs=1) as stpool, \
             tc.tile_pool(name="g2sb", bufs=2) as g2pool, \
             tc.tile_pool(name="tiny", bufs=8) as tinypool, \
             tc.tile_pool(name="zps2", bufs=2, space="PSUM") as zpsum2, \
             tc.tile_pool(name="tps", bufs=2, space="PSUM") as tpsum, \
             tc.tile_pool(name="fps2", bufs=2, space="PSUM") as fpsum2, \
             tc.tile_pool(name="tailps", bufs=2, space="PSUM") as tailpsum:
            stat6 = stpool.tile([128, J2, NNC, 6], f32)
            stat2 = stpool.tile([128, J2, 2], f32)
            for nci in range(NNC):
                ncs = nci * NCH
                ch = load_ch(nci, f"ch2_{nci}")
                for b in range(BC):
                    xg2t = []
                    z1l = []
                    z2l = []
                    rhl = []
                    for jt in range(3):
                        j2 = 3 * b + jt
                        zps = zpsum2.tile([128, NCH], f32, tag="zg2")
                        for mi in range(NT):
                            nc.tensor.matmul(zps[:],
                                             xg1T_sb[:, mi, j2 * 128:(j2 + 1) * 128],
                                             ch[:, 0, mi, :],
                                             start=(mi == 0), stop=(mi == NT - 1))
                        z1t = g2pool.tile([128, NCH], f32r, tag="z1", bufs=3)
                        nc.vector.tensor_copy(z1t[:], zps[:])
                        z1l.append(z1t)
                        zps2 = zpsum2.tile([128, NCH], f32, tag="zg2")
                        for mi in range(NT):
                            nc.tensor.matmul(zps2[:],
                                             xg1T_sb[:, mi, j2 * 128:(j2 + 1) * 128],
                                             ch[:, 1, mi, :],
                                             start=(mi == 0), stop=(mi == NT - 1))
                        z2t = g2pool.tile([128, NCH], f32r, tag="z2", bufs=3)
                        nc.scalar.activation(z2t[:], zps2[:], AF.Copy)
                        z2l.append(z2t)
                        xg1rhs = g2pool.tile([128, NCH], f32r, tag="xg1rhs", bufs=3)
                        tp = tpsum.tile([128, NTL, 128], bf16)
                        for ntl in range(NTL):
                            nc.tensor.transpose(
                                tp[:, ntl, :], xg1T_sb[:, nci * NTL + ntl,
                                                       j2 * 128:(j2 + 1) * 128],
                                identb_sb[:])
                        nc.vector.tensor_copy(
                            xg1rhs[:].rearrange("p (a c) -> p a c", c=128), tp[:])
                        rhl.append(xg1rhs)
                    for jt in range(3):
                        fps = fpsum2.tile([128, NCH], f32)
                        nc.tensor.matmul(fps[:], th2_sb[:, 0:128], rhl[jt][:],
                                         start=True, stop=False)
                        nc.tensor.matmul(fps[:], th2_sb[:, 128:256], z1l[jt][:],
                                         start=False, stop=False)
                        nc.tensor.matmul(fps[:], th2_sb[:, 256:384], z2l[jt][:],
                                         start=False, stop=True)
                        xt = g2pool.tile([128, NCH], f32r, tag="xg2t", bufs=4)
                        nc.scalar.activation(xt[:], fps[:], AF.Relu)
                        xg2t.append(xt)
                        if DEBUG_TAPS:
                            j2_ = 3 * b + jt
                            dma(dbg_xg2_d[j2_ * 128:(j2_ + 1) * 128,
                                          ncs:ncs + NCH], xt[:].bitcast(f32))
                    # residual input tiles for this b (2 distinct jx)
                    xres = {}
                    for jt in range(3):
                        jx = (3 * b + jt) // 2
                        if jx not in xres:
                            xr = g2pool.tile([128, NCH], f32r, tag="xres", bufs=2,
                                             name=f"xres_{nci}_{jx}")
                            dma(xr[:], xb_d[jx * 128:(jx + 1) * 128, ncs:ncs + NCH].bitcast(f32r))
                            xres[jx] = xr
                    # temporal conv 1
                    xt1b = []
                    for jt in range(3):
                        ja, jb = TC_PAIRS[jt]
                        tp1 = tailpsum.tile([128, NCH], f32, tag="tailp")
                        ca = ((0 * 3 + jt) * 3 + ja) * 128
                        cb = ((0 * 3 + jt) * 3 + jb) * 128
                        nc.tensor.matmul(tp1[:], tcbd_sb[:, ca:ca + 128], xg2t[ja][:],
                                         start=True, stop=False)
                        nc.tensor.matmul(tp1[:], tcbd_sb[:, cb:cb + 128], xg2t[jb][:],
                                         start=False, stop=True)
                        x1 = g2pool.tile([128, NCH], f32r, tag="xt1b", bufs=4)
                        nc.scalar.activation(x1[:], tp1[:], AF.Relu,
                                             bias=bias3_sb[:, 0:1])
                        xt1b.append(x1)
                    # temporal conv 2 + residual + y chunk + stats
                    for jt in range(3):
                        j2 = 3 * b + jt
                        ja, jb = TC_PAIRS[jt]
                        tp2 = tailpsum.tile([128, NCH], f32, tag="tailp")
                        ca = ((1 * 3 + jt) * 3 + ja) * 128
                        cb = ((1 * 3 + jt) * 3 + jb) * 128
                        nc.tensor.matmul(tp2[:], tcbd_sb[:, ca:ca + 128], xt1b[ja][:],
                                         start=True, stop=False)
                        nc.tensor.matmul(tp2[:], tcbd_sb[:, cb:cb + 128], xt1b[jb][:],
                                         start=False, stop=True)
                        xt2c = g2pool.tile([128, NCH], f32r, tag="xt2c", bufs=2)
                        nc.scalar.activation(xt2c[:], tp2[:], AF.Relu,
                                             bias=bias3_sb[:, 1:2])
                        jx, half = divmod(j2, 2)
                        rp = tailpsum.tile([128, NCH], f32, tag="tailp")
                        nc.tensor.matmul(rp[:],
                                         resbd_sb[:, half * 128:(half + 1) * 128].bitcast(f32r),
                                         xres[jx][:].bitcast(f32r),
                                         start=True, stop=True)
                        if nci < NNC - 1:
                            ycur = g2pool.tile([128, NCH], f32, tag="ycur", bufs=3)
                            nc.vector.scalar_tensor_tensor(ycur[:],
                                                           rp[:], bias3_sb[:, 2:3],
                                                           xt2c[:].bitcast(f32),
                                                           ALU.add, ALU.add)
                            nc.vector.bn_stats(stat6[:, j2, nci, :], ycur[:])
                            dma(ysc_d[j2 * 128:(j2 + 1) * 128, ncs:ncs + NCH],
                                ycur[:])
                        else:
                            yrow = g2pool.tile([128, N], f32, tag="yrow", bufs=2)
                            dma(yrow[:, 0:ncs], ysc_d[j2 * 128:(j2 + 1) * 128,
                                                      0:ncs])
                            nc.vector.scalar_tensor_tensor(yrow[:, ncs:ncs + NCH],
                                                           rp[:], bias3_sb[:, 2:3],
                                                           xt2c[:].bitcast(f32),
                                                           ALU.add, ALU.add)
                            nc.vector.bn_stats(stat6[:, j2, nci, :],
                                               yrow[:, ncs:ncs + NCH])
                            nc.vector.bn_aggr(stat2[:, j2, :], stat6[:, j2, :, :])
                            varp = tinypool.tile([128, 1], f32, tag="varp")
                            nc.vector.tensor_scalar_add(varp[:],
                                                        stat2[:, j2, 1:2], 1e-5)
                            sd = tinypool.tile([128, 1], f32, tag="sd")
                            nc.scalar.sqrt(sd[:], varp[:])
                            istd = tinypool.tile([128, 1], f32, tag="istd")
                            nc.vector.reciprocal(istd[:], sd[:])
                            negmu = tinypool.tile([128, 1], f32, tag="negmu")
                            nc.vector.scalar_tensor_tensor(negmu[:],
                                                           stat2[:, j2, 0:1], -1.0,
                                                           istd[:], ALU.mult,
                                                           ALU.mult)
                            yout = g2pool.tile([128, N], f32, tag="yout", bufs=2)
                            if ln_affine:
                                nc.scalar.activation(yout[:], yrow[:],
                                                     AF.Copy, bias=negmu[:],
                                                     scale=istd[:])
                                nc.vector.tensor_mul(yout[:], yout[:], lng_sb[:])
                                nc.vector.tensor_add(yout[:], yout[:], lnb_sb[:])
                                nc.scalar.activation(yout[:], yout[:], AF.Relu)
                            else:
                                nc.scalar.activation(yout[:], yrow[:],
                                                     AF.Relu, bias=negmu[:],
                                                     scale=istd[:])
                            dma(y_d[j2 * 128:(j2 + 1) * 128, :], yout[:])

        chp.__exit__(None, None, None)
        xg1Tp.__exit__(None, None, None)

    nc.compile()
    return nc


def _host_prep(inputs):
    x = np.asarray(inputs["x"], np.float32)
    cheb = np.asarray(inputs["cheb"], np.float32)
    theta1 = np.asarray(inputs["theta1"], np.float32)
    theta2 = np.asarray(inputs["theta2"], np.float32)
    mlp1_w = np.asarray(inputs["mlp1_w"], np.float32)
    mlp1_b = np.asarray(inputs["mlp1_b"], np.float32)
    mlp2_w = np.asarray(inputs["mlp2_w"], np.float32)
    mlp2_b = np.asarray(inputs["mlp2_b"], np.float32)
    tc1_w = np.asarray(inputs["tc1_w"], np.float32)
    tc1_b = np.asarray(inputs["tc1_b"], np.float32)
    tc2_w = np.asarray(inputs["tc2_w"], np.float32)
    tc2_b = np.asarray(inputs["tc2_b"], np.float32)
    res_w = np.asarray(inputs["res_w"], np.float32)
    res_b = np.asarray(inputs["res_b"], np.float32)
    ln_g = np.asarray(inputs["ln_g"], np.float32)
    ln_b = np.asarray(inputs["ln_b"], np.float32)

    assert np.array_equal(cheb[0], np.eye(N, dtype=np.float32)), \
        "kernel assumes cheb[0] == I"
    ln_affine = not (np.all(ln_g == 1.0) and np.all(ln_b == 0.0))

    bfc = ml_dtypes.bfloat16
    cheb12 = np.ascontiguousarray(cheb[1:3]).astype(bfc)

    th1 = np.zeros((3, 128, 256), np.float32)
    for g in range(8):
        for k in range(3):
            th1[k, g * 16:(g + 1) * 16, g * 32:(g + 1) * 32] = theta1[k]
    th1 = np.ascontiguousarray(th1.transpose(1, 0, 2).reshape(128, 3 * 256))
    th2 = np.zeros((3, 128, 128), np.float32)
    for g in range(4):
        for k in range(3):
            th2[k, g * 32:(g + 1) * 32, g * 32:(g + 1) * 32] = theta2[k]
    th2 = np.ascontiguousarray(th2.transpose(1, 0, 2).reshape(128, 3 * 128))

    src0 = [10] + list(range(11))
    src1 = [11] + list(range(1, 12))
    tcbd = np.zeros((2, 3, 3, 128, 128), np.float32)
    for ti, w in ((0, tc1_w), (1, tc2_w)):
        for tpp in range(12):
            jt, to = divmod(tpp, 4)
            for srcs, kw in ((src0, 0), (src1, 1)):
                tin = srcs[tpp]
                ji, til = divmod(tin, 4)
                tcbd[ti, jt, ji, til * 32:(til + 1) * 32,
                     to * 32:(to + 1) * 32] += w[:, :, 0, kw].T
    tcbd = np.ascontiguousarray(
        tcbd.reshape(18, 128, 128).transpose(1, 0, 2).reshape(128, 18 * 128))

    resbd = np.zeros((2, 128, 128), np.float32)
    for half in range(2):
        for g4 in range(4):
            g = g4 + 4 * half
            resbd[half, g * 16:(g + 1) * 16,
                  g4 * 32:(g4 + 1) * 32] = res_w[:, :, 0, 0].T
    resbd = np.ascontiguousarray(
        resbd.transpose(1, 0, 2).reshape(128, 2 * 128))

    identb = np.eye(128, dtype=np.float32).astype(bfc)
    selE1 = np.zeros((6, 2, 12, 128), np.float32)
    for j in range(6):
        b0, b1 = B01[j]
        for p in range(128):
            bt = 8 * j + p // 16
            bb, t = divmod(bt, 12)
            selE1[j, 0 if bb == b0 else 1, t, p] = 1.0
    selE1 = np.ascontiguousarray(
        selE1.reshape(12, 12, 128).transpose(1, 0, 2).reshape(12, 12 * 128)
    ).astype(bfc)

    w1aug = np.concatenate([mlp1_w.T, mlp1_b[None]], 0).astype(bfc)
    w2aug = np.concatenate([mlp2_w.T, mlp2_b[None]], 0).astype(bfc)
    p32 = np.arange(128) % 32
    bias3 = np.stack([tc1_b[p32], tc2_b[p32], res_b[p32],
                      np.zeros(128, np.float32)], axis=1).astype(np.float32)

    shared = dict(cheb12=cheb12, th1=th1, th2=th2, tcbd=tcbd, resbd=resbd,
                  identb=identb, selE1=selE1, w1aug=w1aug, w2aug=w2aug,
                  bias3=bias3)
    if ln_affine:
        shared["lng"] = np.ascontiguousarray(
            np.broadcast_to(ln_g, (128, N))).astype(np.float32)
        shared["lnb"] = np.ascontiguousarray(
            np.broadcast_to(ln_b, (128, N))).astype(np.float32)

    in_maps = []
    for c in range(NCORES):
        xc = x[c * BC:(c + 1) * BC]                       # [4, 12, 16, N]
        xT = np.ascontiguousarray(
            xc.transpose(3, 0, 1, 2).reshape(N, R1)).astype(bfc)
        xb = np.ascontiguousarray(xc.reshape(R1, N))
        in_maps.append(dict(shared, xT=xT, xb=xb))
    return in_maps, ln_affine


def kernel(**inputs):
    global _compiled, _compiled_affine
    in_maps, ln_affine = _host_prep(inputs)
    if _compiled is None or _compiled_affine != ln_affine:
        _compiled = _build(ln_affine)
        _compiled_affine = ln_affine
    res = run_bass_kernel_spmd(_compiled, in_maps, list(range(NCORES)))
    y = np.empty((B, T, O, N), np.float32)
    for c in range(NCORES):
        y[c * BC:(c + 1) * BC] = res.results[c]["y"].reshape(BC, T, O, N)
    return y


# revision 3
# speedup vs baseline: 1.5479x; 1.5479x over previous
"""Trainium2 Bass kernel for nn_Branch1_block (gnn_message_passing), v2.

Data-parallel over batch on 8 NeuronCores (4 batches/core). All matmul
operands in bf16 (PSUM accumulation fp32): halves DMA + SBUF vs fp32r at
the same PE rate. T2 Chebyshev matrix precomputed on host (no on-chip
recurrence). SE attention is folded into per-j1 scaled copies of the
theta1 block-diagonal (no per-z scaling on DVE). xg2 never leaves SBUF:
the temporal-conv tail is interleaved per n-chunk into gconv2, and
LayerNorm stats accumulate per-chunk via bn_stats/bn_aggr.
"""
import sys

import numpy as np

try:
    import concourse.bass as bass
except ImportError:  # pragma: no cover - grading env fallback
    for p in ("/root/.axon_site", "/root/.axon_site/_ro/trn_rl_repo",
              "/root/.axon_site/_ro/pypackages", "/opt/trn_rl_repo"):
        if p not in sys.path:
            sys.path.append(p)
    import concourse.bass as bass

from contextlib import ExitStack

import ml_dtypes
import concourse.mybir as mybir
import concourse.tile as tile
from concourse import bacc
from concourse.bass_utils import run_bass_kernel_spmd

B, T, F, O, N, K = 32, 12, 16, 32, 2048, 3
NCORES = 8
BC = B // NCORES          # 4 batches per core
BT = BC * T               # 48
R1 = BT * F               # 768 rows (bt,f)
R2 = BT * O               # 1536 rows (bt,o)
J1 = R1 // 128            # 6
J2 = R2 // 128            # 12
NT = N // 128             # 16
NCH = 512
NNC = N // NCH            # 4
NTL = NCH // 128          # 4 n-tiles per chunk

f32 = mybir.dt.float32
f32r = mybir.dt.float32r
bf16 = mybir.dt.bfloat16
AF = mybir.ActivationFunctionType
AX = mybir.AxisListType
ALU = mybir.AluOpType

# (jt, ji) pairs with nonzero temporal-conv block matrices
TC_PAIRS = {0: (0, 2), 1: (0, 1), 2: (1, 2)}
# per j1 (r1 tile): the one or two batches its rows touch
B01 = []
for _j in range(6):
    _bs = sorted({(8 * _j + _p // 16) // 12 for _p in range(128)})
    B01.append((_bs[0], _bs[-1]))

_compiled = None
_compiled_affine = None


DEBUG_TAPS = False


def _build(ln_affine):
    nc = bacc.Bacc("TRN2", target_bir_lowering=False, debug=False)

    xT_d = nc.dram_tensor("xT", [N, R1], bf16, kind="ExternalInput").ap()
    xb_d = nc.dram_tensor("xb", [R1, N], f32, kind="ExternalInput").ap()
    cheb_d = nc.dram_tensor("cheb12", [2, N, N], bf16, kind="ExternalInput").ap()
    th1_d = nc.dram_tensor("th1", [128, 3 * 256], f32, kind="ExternalInput").ap()
    th2_d = nc.dram_tensor("th2", [128, 3 * 128], f32, kind="ExternalInput").ap()
    tcbd_d = nc.dram_tensor("tcbd", [128, 18 * 128], f32, kind="ExternalInput").ap()
    resbd_d = nc.dram_tensor("resbd", [128, 2 * 128], f32, kind="ExternalInput").ap()
    identb_d = nc.dram_tensor("identb", [128, 128], bf16, kind="ExternalInput").ap()
    selE1_d = nc.dram_tensor("selE1", [12, 12 * 128], bf16, kind="ExternalInput").ap()
    w1aug_d = nc.dram_tensor("w1aug", [13, 3], bf16, kind="ExternalInput").ap()
    w2aug_d = nc.dram_tensor("w2aug", [4, 12], bf16, kind="ExternalInput").ap()
    bias3_d = nc.dram_tensor("bias3", [128, 4], f32, kind="ExternalInput").ap()
    if ln_affine:
        lng_d = nc.dram_tensor("lng", [128, N], f32, kind="ExternalInput").ap()
        lnb_d = nc.dram_tensor("lnb", [128, N], f32, kind="ExternalInput").ap()
    y_d = nc.dram_tensor("y", [R2, N], f32, kind="ExternalOutput").ap()
    ysc_d = nc.dram_tensor("ysc", [R2, N], f32).ap()
    if DEBUG_TAPS:
        dbg_att_d = nc.dram_tensor("dbg_att", [128, J1], f32,
                                   kind="ExternalOutput").ap()
        dbg_xg1_d = nc.dram_tensor("dbg_xg1", [128, NT * R2], bf16,
                                   kind="ExternalOutput").ap()
        dbg_xg2_d = nc.dram_tensor("dbg_xg2", [R2, N], f32,
                                   kind="ExternalOutput").ap()

    dma = nc.sync.dma_start

    with tile.TileContext(nc) as tc, ExitStack() as top:
        cpool = top.enter_context(tc.tile_pool(name="const", bufs=1))
        th2_sb = cpool.tile([128, 3 * 128], f32r)
        tcbd_sb = cpool.tile([128, 18 * 128], f32r)
        resbd_sb = cpool.tile([128, 2 * 128], f32r)
        identb_sb = cpool.tile([128, 128], bf16)
        bias3_sb = cpool.tile([128, 4], f32)
        attc1 = cpool.tile([128, J1], f32)
        cdma = nc.scalar.dma_start
        cdma(th2_sb[:], th2_d.bitcast(f32r))
        cdma(tcbd_sb[:], tcbd_d.bitcast(f32r))
        cdma(resbd_sb[:], resbd_d.bitcast(f32r))
        cdma(identb_sb[:], identb_d)
        cdma(bias3_sb[:], bias3_d)

        xg1Tp = tc.tile_pool(name="xg1T", bufs=1)
        xg1Tpool = xg1Tp.__enter__()
        xg1T_sb = xg1Tpool.tile([128, NT, R2], bf16)

        chp = tc.tile_pool(name="chp", bufs=2)
        chpool = chp.__enter__()

        def load_ch(nci, name):
            ncs = nci * NCH
            ch = chpool.tile([128, 2, NT, NCH], bf16, tag="ch", bufs=2, name=name)
            chv = cheb_d[:, :, ncs:ncs + NCH].rearrange(
                "t (mi p) n -> t mi p n", p=128)
            for t_ in range(2):
                for q in range(2):
                    dma(ch[:, t_, q * 8:(q + 1) * 8, :],
                        chv[t_, q * 8:(q + 1) * 8].rearrange("mi p n -> p mi n"))
            return ch

        # ---------- phase A+B: attention + gconv1 ----------
        with tc.tile_pool(name="xTp", bufs=1) as xTpool, \
             tc.tile_pool(name="g1sb", bufs=2) as g1pool, \
             tc.tile_pool(name="attps", bufs=2, space="PSUM") as apsum, \
             tc.tile_pool(name="attsb", bufs=2) as aspool, \
             tc.tile_pool(name="zps", bufs=3, space="PSUM") as zpsum, \
             tc.tile_pool(name="fps", bufs=3, space="PSUM") as fpsum:
            xTq = [xTpool.tile([128, 4, R1], bf16, name=f"xTq{q}")
                   for q in range(4)]
            th1_sb = xTpool.tile([128, 3 * 256], f32)
            th1s_sb = xTpool.tile([128, J1, 3 * 256], f32r)
            nc.scalar.dma_start(th1_sb[:], th1_d)
            xTv = xT_d.rearrange("(mi p) r -> mi p r", p=128)
            for q in range(4):
                dma(xTq[q][:], xTv[q * 4:(q + 1) * 4].rearrange("mi p r -> p mi r"))

            def xT_mi(mi):
                return xTq[mi // 4][:, mi % 4, :]
            ch0 = load_ch(0, "ch_0")
            ch1 = load_ch(1, "ch_1")

            # ---- SE attention ----
            ones_col = aspool.tile([128, 1], bf16, tag="ones")
            nc.vector.memset(ones_col[:], 1.0)
            rs48 = aspool.tile([1, BT], f32, tag="rs48")
            ident1 = aspool.tile([1, 1], f32, tag="ident1")
            nc.vector.memset(ident1[:], 1.0)
            for h in range(2):
                attps = apsum.tile([1, R1 // 2], f32, tag="attp")
                for mi in range(NT):
                    nc.tensor.matmul(attps[:], ones_col[:],
                                     xT_mi(mi)[:, h * 384:(h + 1) * 384],
                                     start=(mi == 0), stop=(mi == NT - 1))
                nc.vector.reduce_sum(rs48[:, h * 24:(h + 1) * 24],
                                     attps[:].rearrange("p (a b) -> p a b", b=F),
                                     axis=AX.X)
            t48ps = apsum.tile([BT, 1], f32, tag="attp")
            nc.tensor.transpose(t48ps[:], rs48[:], ident1[:])
            att0sb = aspool.tile([BT, 1], bf16, tag="att0")
            nc.scalar.activation(att0sb[:], t48ps[:], AF.Copy, scale=1.0 / (F * N))
            atbps = apsum.tile([12, 4], f32, tag="attp")
            for b in range(4):
                nc.tensor.matmul(atbps[:, b:b + 1],
                                 identb_sb[:48, b * 12:(b + 1) * 12],
                                 att0sb[:], start=True, stop=True)
            atb13 = aspool.tile([13, 4], bf16, tag="atb13")
            nc.vector.memset(atb13[:], 1.0)
            nc.scalar.activation(atb13[:12, :], atbps[:], AF.Copy)
            w1aug_sb = aspool.tile([13, 3], bf16, tag="w1aug")
            w2aug_sb = aspool.tile([4, 12], bf16, tag="w2aug")
            selE1_sb = aspool.tile([12, 12 * 128], bf16, tag="selE1")
            nc.scalar.dma_start(w1aug_sb[:], w1aug_d)
            nc.scalar.dma_start(w2aug_sb[:], w2aug_d)
            nc.scalar.dma_start(selE1_sb[:], selE1_d)
            a1ps = apsum.tile([3, 4], f32, tag="attp")
            nc.tensor.matmul(a1ps[:], w1aug_sb[:], atb13[:], start=True, stop=True)
            a1sb = aspool.tile([4, 4], bf16, tag="a1")
            nc.vector.memset(a1sb[:], 1.0)
            nc.scalar.activation(a1sb[:3, :], a1ps[:], AF.Relu)
            attps2 = apsum.tile([12, 4], f32, tag="attp")
            nc.tensor.matmul(attps2[:], w2aug_sb[:], a1sb[:], start=True, stop=True)
            att_tb = aspool.tile([12, 4], bf16, tag="att_tb")
            nc.scalar.activation(att_tb[:], attps2[:], AF.Sigmoid)
            for j in range(J1):
                b0, b1 = B01[j]
                acps = apsum.tile([128, 1], f32, tag="attp")
                nc.tensor.matmul(acps[:], selE1_sb[:, (j * 2) * 128:(j * 2 + 1) * 128],
                                 att_tb[:, b0:b0 + 1], start=True, stop=False)
                nc.tensor.matmul(acps[:], selE1_sb[:, (j * 2 + 1) * 128:(j * 2 + 2) * 128],
                                 att_tb[:, b1:b1 + 1], start=False, stop=True)
                nc.scalar.activation(attc1[:, j:j + 1], acps[:], AF.Copy)
                nc.vector.tensor_scalar_mul(th1s_sb[:, j, :], th1_sb[:],
                                            attc1[:, j:j + 1])

            # ---- gconv1: graph + feature per n-chunk ----
            chq = [ch0, ch1]
            for nci in range(NNC):
                ncs = nci * NCH
                ch = chq[nci] if nci < 2 else load_ch(nci, f"ch_{nci}")
                z0s = []
                for j1 in range(J1):
                    z0t = g1pool.tile([128, NCH], f32r, tag="z0", bufs=7,
                                      name=f"z0_{nci}_{j1}")
                    dma(z0t[:], xb_d[j1 * 128:(j1 + 1) * 128, ncs:ncs + NCH].bitcast(f32r))
                    z0s.append(z0t)

                def g1_feat(j1, zs):
                    z0t, z1t, z2t = zs
                    for ntl in range(NTL):
                        fps = fpsum.tile([128, 256], f32, tag="fps")
                        nc.tensor.matmul(fps[:],
                                         z0t[:, ntl * 128:(ntl + 1) * 128],
                                         th1s_sb[:, j1, 0:256],
                                         start=True, stop=False)
                        nc.tensor.matmul(fps[:],
                                         z1t[:, ntl * 128:(ntl + 1) * 128],
                                         th1s_sb[:, j1, 256:512],
                                         start=False, stop=False)
                        nc.tensor.matmul(fps[:],
                                         z2t[:, ntl * 128:(ntl + 1) * 128],
                                         th1s_sb[:, j1, 512:768],
                                         start=False, stop=True)
                        nc.scalar.activation(
                            xg1T_sb[:, nci * NTL + ntl, j1 * 256:(j1 + 1) * 256],
                            fps[:], AF.Relu)

                prev = None
                for j1 in range(J1):
                    zcur = [z0s[j1]]
                    for k in (1, 2):
                        zps = zpsum.tile([128, NCH], f32)
                        for mi in range(NT):
                            nc.tensor.matmul(zps[:],
                                             xT_mi(mi)[:, j1 * 128:(j1 + 1) * 128],
                                             ch[:, k - 1, mi, :],
                                             start=(mi == 0), stop=(mi == NT - 1))
                        zt = g1pool.tile([128, NCH], f32r, tag=f"z{k}", bufs=3)
                        if k == 1:
                            nc.vector.tensor_copy(zt[:], zps[:])
                        else:
                            nc.scalar.activation(zt[:], zps[:], AF.Copy)
                        zcur.append(zt)
                    if prev is not None:
                        g1_feat(j1 - 1, prev)
                    prev = zcur
                g1_feat(J1 - 1, prev)

        if DEBUG_TAPS:
            dma(dbg_att_d, attc1[:])
            dma(dbg_xg1_d.rearrange("p (mi r) -> p mi r", r=R2), xg1T_sb[:])

        # ---------- phase C: gconv2 + temporal tail + LayerNorm ----------
        with tc.tile_pool(name="stat", bufs=1) as stpool, \
             tc.tile_pool(name="g2sb", bufs=2) as g2pool, \
             tc.tile_pool(name="tiny", bufs=8) as tinypool, \
             tc.tile_pool(name="zps2", bufs=2, space="PSUM") as zpsum2, \
             tc.tile_pool(name="tps", bufs=2, space="PSUM") as tpsum, \
             tc.tile_pool(name="fps2", bufs=2, space="PSUM") as fpsum2, \
             tc.tile_pool(name="tailps", bufs=2, space="PSUM") as tailpsum:
            stat6 = stpool.tile([128, J2, NNC, 6], f32)
            stat2 = stpool.tile([128, J2, 2], f32)
            for nci in range(NNC):
                ncs = nci * NCH
                ch = load_ch(nci, f"ch2_{nci}")
                for b in range(BC):
                    xg2t = []
                    z1l = []
                    z2l = []
                    rhl = []
                    for jt in range(3):
                        j2 = 3 * b + jt
                        zps = zpsum2.tile([128, NCH], f32, tag="zg2")
                        for mi in range(NT):
                            nc.tensor.matmul(zps[:],
                                             xg1T_sb[:, mi, j2 * 128:(j2 + 1) * 128],
                                             ch[:, 0, mi, :],
                                             start=(mi == 0), stop=(mi == NT - 1))
                        z1t = g2pool.tile([128, NCH], f32r, tag="z1", bufs=3)
                        nc.vector.tensor_copy(z1t[:], zps[:])
                        z1l.append(z1t)
                        zps2 = zpsum2.tile([128, NCH], f32, tag="zg2")
                        for mi in range(NT):
                            nc.tensor.matmul(zps2[:],
                                             xg1T_sb[:, mi, j2 * 128:(j2 + 1) * 128],
                                             ch[:, 1, mi, :],
                                             start=(mi == 0), stop=(mi == NT - 1))
                        z2t = g2pool.tile([128, NCH], f32r, tag="z2", bufs=3)
                        nc.scalar.activation(z2t[:], zps2[:], AF.Copy)
                        z2l.append(z2t)
                        xg1rhs = g2pool.tile([128, NCH], f32r, tag="xg1rhs", bufs=3)
                        tp = tpsum.tile([128, NTL, 128], bf16)
                        for ntl in range(NTL):
                            nc.tensor.transpose(
                                tp[:, ntl, :], xg1T_sb[:, nci * NTL + ntl,
                                                       j2 * 128:(j2 + 1) * 128],
                                identb_sb[:])
                        nc.vector.tensor_copy(
                            xg1rhs[:].rearrange("p (a c) -> p a c", c=128), tp[:])
                        rhl.append(xg1rhs)
                    for jt in range(3):
                        fps = fpsum2.tile([128, NCH], f32)
                        nc.tensor.matmul(fps[:], th2_sb[:, 0:128], rhl[jt][:],
                                         start=True, stop=False)
                        nc.tensor.matmul(fps[:], th2_sb[:, 128:256], z1l[jt][:],
                                         start=False, stop=False)
                        nc.tensor.matmul(fps[:], th2_sb[:, 256:384], z2l[jt][:],
                                         start=False, stop=True)
                        xt = g2pool.tile([128, NCH], f32r, tag="xg2t", bufs=4)
                        nc.scalar.activation(xt[:], fps[:], AF.Relu)
                        xg2t.append(xt)
                        if DEBUG_TAPS:
                            j2_ = 3 * b + jt
                            dma(dbg_xg2_d[j2_ * 128:(j2_ + 1) * 128,
                                          ncs:ncs + NCH], xt[:].bitcast(f32))
                    # residual input tiles for this b (2 distinct jx)
                    xres = {}
                    for jt in range(3):
                        jx = (3 * b + jt) // 2
                        if jx not in xres:
                            xr = g2pool.tile([128, NCH], f32r, tag="xres", bufs=2,
                                             name=f"xres_{nci}_{jx}")
                            dma(xr[:], xb_d[jx * 128:(jx + 1) * 128, ncs:ncs + NCH].bitcast(f32r))
                            xres[jx] = xr
                    # temporal conv 1
                    xt1b = []
                    for jt in range(3):
                        ja, jb = TC_PAIRS[jt]
                        tp1 = tailpsum.tile([128, NCH], f32, tag="tailp")
                        ca = ((0 * 3 + jt) * 3 + ja) * 128
                        cb = ((0 * 3 + jt) * 3 + jb) * 128
                        nc.tensor.matmul(tp1[:], tcbd_sb[:, ca:ca + 128], xg2t[ja][:],
                                         start=True, stop=False)
                        nc.tensor.matmul(tp1[:], tcbd_sb[:, cb:cb + 128], xg2t[jb][:],
                                         start=False, stop=True)
                        x1 = g2pool.tile([128, NCH], f32r, tag="xt1b", bufs=4)
                        nc.scalar.activation(x1[:], tp1[:], AF.Relu,
                                             bias=bias3_sb[:, 0:1])
                        xt1b.append(x1)
                    # temporal conv 2 + residual + y chunk + stats
                    for jt in range(3):
                        j2 = 3 * b + jt
                        ja, jb = TC_PAIRS[jt]
                        tp2 = tailpsum.tile([128, NCH], f32, tag="tailp")
                        ca = ((1 * 3 + jt) * 3 + ja) * 128
                        cb = ((1 * 3 + jt) * 3 + jb) * 128
                        nc.tensor.matmul(tp2[:], tcbd_sb[:, ca:ca + 128], xt1b[ja][:],
                                         start=True, stop=False)
                        nc.tensor.matmul(tp2[:], tcbd_sb[:, cb:cb + 128], xt1b[jb][:],
                                         start=False, stop=True)
                        xt2c = g2pool.tile([128, NCH], f32r, tag="xt2c", bufs=2)
                        nc.scalar.activation(xt2c[:], tp2[:], AF.Relu,
                                             bias=bias3_sb[:, 1:2])
                        jx, half = divmod(j2, 2)
                        rp = tailpsum.tile([128, NCH], f32, tag="tailp")
                        nc.tensor.matmul(rp[:],
                                         resbd_sb[:, half * 128:(half + 1) * 128].bitcast(f32r),
                                         xres[jx][:].bitcast(f32r),
                                         start=True, stop=True)
                        if nci < NNC - 1:
                            ycur = g2pool.tile([128, NCH], f32, tag="ycur", bufs=3)
                            nc.vector.scalar_tensor_tensor(ycur[:],
                                                           rp[:], bias3_sb[:, 2:3],
                                                           xt2c[:].bitcast(f32),
                                                           ALU.add, ALU.add)
                            nc.vector.bn_stats(stat6[:, j2, nci, :], ycur[:])
                            dma(ysc_d[j2 * 128:(j2 + 1) * 128, ncs:ncs + NCH],
                                ycur[:])
                        else:
                            yrow = g2pool.tile([128, N], f32, tag="yrow", bufs=2)
                            dma(yrow[:, 0:ncs], ysc_d[j2 * 128:(j2 + 1) * 128,
                                                      0:ncs])
                            nc.vector.scalar_tensor_tensor(yrow[:, ncs:ncs + NCH],
                                                           rp[:], bias3_sb[:, 2:3],
                                                           xt2c[:].bitcast(f32),
                                                           ALU.add, ALU.add)
                            nc.vector.bn_stats(stat6[:, j2, nci, :],
                                               yrow[:, ncs:ncs + NCH])
                            nc.vector.bn_aggr(stat2[:, j2, :], stat6[:, j2, :, :])
                            varp = tinypool.tile([128, 1], f32, tag="varp")
                            nc.vector.tensor_scalar_add(varp[:],
                                                        stat2[:, j2, 1:2], 1e-5)
                            sd = tinypool.tile([128, 1], f32, tag="sd")
                            nc.scalar.sqrt(sd[:], varp[:])
                            istd = tinypool.tile([128, 1], f32, tag="istd")
                            nc.vector.reciprocal(istd[:], sd[:])
                            negmu = tinypool.tile([128, 1], f32, tag="negmu")
                            nc.vector.scalar_tensor_tensor(negmu[:],
                                                           stat2[:, j2, 0:1], -1.0,
                                                           istd[:], ALU.mult,
                                                           ALU.mult)
                            for c in range(NNC):
                                cs = c * NCH
                                yo = g2pool.tile([128, NCH], f32, tag="yout",
                                                 bufs=4)
                                if ln_affine:
                                    lgt = g2pool.tile([128, NCH], f32,
                                                      tag="lgt", bufs=2)
                                    lbt = g2pool.tile([128, NCH], f32,
                                                      tag="lbt", bufs=2)
                                    dma(lgt[:], lng_d[:, cs:cs + NCH])
                                    dma(lbt[:], lnb_d[:, cs:cs + NCH])
                                    nc.vector.tensor_scalar(yo[:],
                                                            yrow[:, cs:cs + NCH],
                                                            istd[:], negmu[:],
                                                            ALU.mult, ALU.add)
                                    nc.vector.tensor_mul(yo[:], yo[:], lgt[:])
                                    nc.vector.tensor_add(yo[:], yo[:], lbt[:])
                                    nc.scalar.activation(yo[:], yo[:], AF.Relu)
                                elif c % 2 == 0:
                                    nc.scalar.activation(yo[:],
                                                         yrow[:, cs:cs + NCH],
                                                         AF.Relu, bias=negmu[:],
                                                         scale=istd[:])
                                else:
                                    nc.vector.tensor_scalar(yo[:],
                                                            yrow[:, cs:cs + NCH],
                                                            istd[:], negmu[:],
                                                            ALU.mult, ALU.add)
                                    nc.vector.tensor_relu(yo[:], yo[:])
                                dma(y_d[j2 * 128:(j2 + 1) * 128, cs:cs + NCH],
                                    yo[:])

        chp.__exit__(None, None, None)
        xg1Tp.__exit__(None, None, None)

    nc.compile()
    return nc


def _host_prep(inputs):
    x = np.asarray(inputs["x"], np.float32)
    cheb = np.asarray(inputs["cheb"], np.float32)
    theta1 = np.asarray(inputs["theta1"], np.float32)
    theta2 = np.asarray(inputs["theta2"], np.float32)
    mlp1_w = np.asarray(inputs["mlp1_w"], np.float32)
    mlp1_b = np.asarray(inputs["mlp1_b"], np.float32)
    mlp2_w = np.asarray(inputs["mlp2_w"], np.float32)
    mlp2_b = np.asarray(inputs["mlp2_b"], np.float32)
    tc1_w = np.asarray(inputs["tc1_w"], np.float32)
    tc1_b = np.asarray(inputs["tc1_b"], np.float32)
    tc2_w = np.asarray(inputs["tc2_w"], np.float32)
    tc2_b = np.asarray(inputs["tc2_b"], np.float32)
    res_w = np.asarray(inputs["res_w"], np.float32)
    res_b = np.asarray(inputs["res_b"], np.float32)
    ln_g = np.asarray(inputs["ln_g"], np.float32)
    ln_b = np.asarray(inputs["ln_b"], np.float32)

    assert np.array_equal(cheb[0], np.eye(N, dtype=np.float32)), \
        "kernel assumes cheb[0] == I"
    ln_affine = not (np.all(ln_g == 1.0) and np.all(ln_b == 0.0))

    bfc = ml_dtypes.bfloat16
    cheb12 = np.ascontiguousarray(cheb[1:3]).astype(bfc)

    th1 = np.zeros((3, 128, 256), np.float32)
    for g in range(8):
        for k in range(3):
            th1[k, g * 16:(g + 1) * 16, g * 32:(g + 1) * 32] = theta1[k]
    th1 = np.ascontiguousarray(th1.transpose(1, 0, 2).reshape(128, 3 * 256))
    th2 = np.zeros((3, 128, 128), np.float32)
    for g in range(4):
        for k in range(3):
            th2[k, g * 32:(g + 1) * 32, g * 32:(g + 1) * 32] = theta2[k]
    th2 = np.ascontiguousarray(th2.transpose(1, 0, 2).reshape(128, 3 * 128))

    src0 = [10] + list(range(11))
    src1 = [11] + list(range(1, 12))
    tcbd = np.zeros((2, 3, 3, 128, 128), np.float32)
    for ti, w in ((0, tc1_w), (1, tc2_w)):
        for tpp in range(12):
            jt, to = divmod(tpp, 4)
            for srcs, kw in ((src0, 0), (src1, 1)):
                tin = srcs[tpp]
                ji, til = divmod(tin, 4)
                tcbd[ti, jt, ji, til * 32:(til + 1) * 32,
                     to * 32:(to + 1) * 32] += w[:, :, 0, kw].T
    tcbd = np.ascontiguousarray(
        tcbd.reshape(18, 128, 128).transpose(1, 0, 2).reshape(128, 18 * 128))

    resbd = np.zeros((2, 128, 128), np.float32)
    for half in range(2):
        for g4 in range(4):
            g = g4 + 4 * half
            resbd[half, g * 16:(g + 1) * 16,
                  g4 * 32:(g4 + 1) * 32] = res_w[:, :, 0, 0].T
    resbd = np.ascontiguousarray(
        resbd.transpose(1, 0, 2).reshape(128, 2 * 128))

    identb = np.eye(128, dtype=np.float32).astype(bfc)
    selE1 = np.zeros((6, 2, 12, 128), np.float32)
    for j in range(6):
        b0, b1 = B01[j]
        for p in range(128):
            bt = 8 * j + p // 16
            bb, t = divmod(bt, 12)
            selE1[j, 0 if bb == b0 else 1, t, p] = 1.0
    selE1 = np.ascontiguousarray(
        selE1.reshape(12, 12, 128).transpose(1, 0, 2).reshape(12, 12 * 128)
    ).astype(bfc)

    w1aug = np.concatenate([mlp1_w.T, mlp1_b[None]], 0).astype(bfc)
    w2aug = np.concatenate([mlp2_w.T, mlp2_b[None]], 0).astype(bfc)
    p32 = np.arange(128) % 32
    bias3 = np.stack([tc1_b[p32], tc2_b[p32], res_b[p32],
                      np.zeros(128, np.float32)], axis=1).astype(np.float32)

    shared = dict(cheb12=cheb12, th1=th1, th2=th2, tcbd=tcbd, resbd=resbd,
                  identb=identb, selE1=selE1, w1aug=w1aug, w2aug=w2aug,
                  bias3=bias3)
    if ln_affine:
        shared["lng"] = np.ascontiguousarray(
            np.broadcast_to(ln_g, (128, N))).astype(np.float32)
        shared["lnb"] = np.ascontiguousarray(
            np.broadcast_to(ln_b, (128, N))).astype(np.float32)

    in_maps = []
    for c in range(NCORES):
        xc = x[c * BC:(c + 1) * BC]                       # [4, 12, 16, N]
        xT = np.ascontiguousarray(
            xc.transpose(3, 0, 1, 2).reshape(N, R1)).astype(bfc)
        xb = np.ascontiguousarray(xc.reshape(R1, N))
        in_maps.append(dict(shared, xT=xT, xb=xb))
    return in_maps, ln_affine


def kernel(**inputs):
    global _compiled, _compiled_affine
    in_maps, ln_affine = _host_prep(inputs)
    if _compiled is None or _compiled_affine != ln_affine:
        _compiled = _build(ln_affine)
        _compiled_affine = ln_affine
    res = run_bass_kernel_spmd(_compiled, in_maps, list(range(NCORES)))
    y = np.empty((B, T, O, N), np.float32)
    for c in range(NCORES):
        y[c * BC:(c + 1) * BC] = res.results[c]["y"].reshape(BC, T, O, N)
    return y


# revision 4
# speedup vs baseline: 1.5616x; 1.0089x over previous
"""Trainium2 Bass kernel for nn_Branch1_block (gnn_message_passing), v2.

Data-parallel over batch on 8 NeuronCores (4 batches/core). All matmul
operands in bf16 (PSUM accumulation fp32): halves DMA + SBUF vs fp32r at
the same PE rate. T2 Chebyshev matrix precomputed on host (no on-chip
recurrence). SE attention is folded into per-j1 scaled copies of the
theta1 block-diagonal (no per-z scaling on DVE). xg2 never leaves SBUF:
the temporal-conv tail is interleaved per n-chunk into gconv2, and
LayerNorm stats accumulate per-chunk via bn_stats/bn_aggr.
"""
import sys

import numpy as np

try:
    import concourse.bass as bass
except ImportError:  # pragma: no cover - grading env fallback
    for p in ("/root/.axon_site", "/root/.axon_site/_ro/trn_rl_repo",
              "/root/.axon_site/_ro/pypackages", "/opt/trn_rl_repo"):
        if p not in sys.path:
            sys.path.append(p)
    import concourse.bass as bass

from contextlib import ExitStack

import ml_dtypes
import concourse.mybir as mybir
import concourse.tile as tile
from concourse import bacc
from concourse.bass_utils import run_bass_kernel_spmd

B, T, F, O, N, K = 32, 12, 16, 32, 2048, 3
NCORES = 8
BC = B // NCORES          # 4 batches per core
BT = BC * T               # 48
R1 = BT * F               # 768 rows (bt,f)
R2 = BT * O               # 1536 rows (bt,o)
J1 = R1 // 128            # 6
J2 = R2 // 128            # 12
NT = N // 128             # 16
NCH = 512
NNC = N // NCH            # 4
NTL = NCH // 128          # 4 n-tiles per chunk

f32 = mybir.dt.float32
f32r = mybir.dt.float32r
bf16 = mybir.dt.bfloat16
AF = mybir.ActivationFunctionType
AX = mybir.AxisListType
ALU = mybir.AluOpType

# (jt, ji) pairs with nonzero temporal-conv block matrices
TC_PAIRS = {0: (0, 2), 1: (0, 1), 2: (1, 2)}
# per j1 (r1 tile): the one or two batches its rows touch
B01 = []
for _j in range(6):
    _bs = sorted({(8 * _j + _p // 16) // 12 for _p in range(128)})
    B01.append((_bs[0], _bs[-1]))

_compiled = None
_compiled_affine = None


DEBUG_TAPS = False


def _build(ln_affine, t2diag):
    nc = bacc.Bacc("TRN2", target_bir_lowering=False, debug=False)

    xT_d = nc.dram_tensor("xT", [N, R1], bf16, kind="ExternalInput").ap()
    xb_d = nc.dram_tensor("xb", [R1, N], f32, kind="ExternalInput").ap()
    if t2diag:
        cheb_d = nc.dram_tensor("cheb1", [N, N], bf16, kind="ExternalInput").ap()
        t2bc_d = nc.dram_tensor("t2bc", [128, N], f32, kind="ExternalInput").ap()
    else:
        cheb_d = nc.dram_tensor("cheb12", [2, N, N], bf16,
                                kind="ExternalInput").ap()
    th1_d = nc.dram_tensor("th1", [128, 3 * 256], f32, kind="ExternalInput").ap()
    th2_d = nc.dram_tensor("th2", [128, 3 * 128], f32, kind="ExternalInput").ap()
    tcbd_d = nc.dram_tensor("tcbd", [128, 18 * 128], f32, kind="ExternalInput").ap()
    resbd_d = nc.dram_tensor("resbd", [128, 2 * 128], f32, kind="ExternalInput").ap()
    identb_d = nc.dram_tensor("identb", [128, 128], bf16, kind="ExternalInput").ap()
    selE1_d = nc.dram_tensor("selE1", [12, 12 * 128], bf16, kind="ExternalInput").ap()
    w1aug_d = nc.dram_tensor("w1aug", [13, 3], bf16, kind="ExternalInput").ap()
    w2aug_d = nc.dram_tensor("w2aug", [4, 12], bf16, kind="ExternalInput").ap()
    bias3_d = nc.dram_tensor("bias3", [128, 4], f32, kind="ExternalInput").ap()
    if ln_affine:
        lng_d = nc.dram_tensor("lng", [128, N], f32, kind="ExternalInput").ap()
        lnb_d = nc.dram_tensor("lnb", [128, N], f32, kind="ExternalInput").ap()
    y_d = nc.dram_tensor("y", [R2, N], f32, kind="ExternalOutput").ap()
    ysc_d = nc.dram_tensor("ysc", [R2, N], f32).ap()
    if DEBUG_TAPS:
        dbg_att_d = nc.dram_tensor("dbg_att", [128, J1], f32,
                                   kind="ExternalOutput").ap()
        dbg_xg1_d = nc.dram_tensor("dbg_xg1", [128, NT * R2], bf16,
                                   kind="ExternalOutput").ap()
        dbg_xg2_d = nc.dram_tensor("dbg_xg2", [R2, N], f32,
                                   kind="ExternalOutput").ap()

    dma = nc.sync.dma_start

    with tile.TileContext(nc) as tc, ExitStack() as top:
        cpool = top.enter_context(tc.tile_pool(name="const", bufs=1))
        th2_sb = cpool.tile([128, 3 * 128], f32r)
        if t2diag:
            t2bc_sb = cpool.tile([128, N], f32)
        tcbd_sb = cpool.tile([128, 18 * 128], f32r)
        resbd_sb = cpool.tile([128, 2 * 128], f32r)
        identb_sb = cpool.tile([128, 128], bf16)
        bias3_sb = cpool.tile([128, 4], f32)
        attc1 = cpool.tile([128, J1], f32)
        cdma = nc.scalar.dma_start
        if t2diag:
            cdma(t2bc_sb[:], t2bc_d)
        cdma(th2_sb[:], th2_d.bitcast(f32r))
        cdma(tcbd_sb[:], tcbd_d.bitcast(f32r))
        cdma(resbd_sb[:], resbd_d.bitcast(f32r))
        cdma(identb_sb[:], identb_d)
        cdma(bias3_sb[:], bias3_d)

        xg1Tp = tc.tile_pool(name="xg1T", bufs=1)
        xg1Tpool = xg1Tp.__enter__()
        xg1T_sb = xg1Tpool.tile([128, NT, R2], bf16)

        chp = tc.tile_pool(name="chp", bufs=2)
        chpool = chp.__enter__()

        def load_ch(nci, name):
            ncs = nci * NCH
            if t2diag:
                ch = chpool.tile([128, NT, NCH], bf16, tag="ch", bufs=2,
                                 name=name)
                chv = cheb_d[:, ncs:ncs + NCH].rearrange(
                    "(mi p) n -> mi p n", p=128)
                for q in range(2):
                    dma(ch[:, q * 8:(q + 1) * 8, :],
                        chv[q * 8:(q + 1) * 8].rearrange("mi p n -> p mi n"))
                return ch
            ch = chpool.tile([128, 2, NT, NCH], bf16, tag="ch", bufs=2, name=name)
            chv = cheb_d[:, :, ncs:ncs + NCH].rearrange(
                "t (mi p) n -> t mi p n", p=128)
            for t_ in range(2):
                for q in range(2):
                    dma(ch[:, t_, q * 8:(q + 1) * 8, :],
                        chv[t_, q * 8:(q + 1) * 8].rearrange("mi p n -> p mi n"))
            return ch

        def ch_k(ch, k, mi):
            return ch[:, mi, :] if t2diag else ch[:, k - 1, mi, :]

        # ---------- phase A+B: attention + gconv1 ----------
        with tc.tile_pool(name="xTp", bufs=1) as xTpool, \
             tc.tile_pool(name="g1sb", bufs=2) as g1pool, \
             tc.tile_pool(name="attps", bufs=2, space="PSUM") as apsum, \
             tc.tile_pool(name="attsb", bufs=2) as aspool, \
             tc.tile_pool(name="zps", bufs=3, space="PSUM") as zpsum, \
             tc.tile_pool(name="fps", bufs=3, space="PSUM") as fpsum:
            xTq = [xTpool.tile([128, 4, R1], bf16, name=f"xTq{q}")
                   for q in range(4)]
            th1_sb = xTpool.tile([128, 3 * 256], f32)
            th1s_sb = xTpool.tile([128, J1, 3 * 256], f32r)
            nc.scalar.dma_start(th1_sb[:], th1_d)
            xTv = xT_d.rearrange("(mi p) r -> mi p r", p=128)
            for q in range(4):
                dma(xTq[q][:], xTv[q * 4:(q + 1) * 4].rearrange("mi p r -> p mi r"))

            def xT_mi(mi):
                return xTq[mi // 4][:, mi % 4, :]
            ch0 = load_ch(0, "ch_0")
            ch1 = load_ch(1, "ch_1")

            # ---- SE attention ----
            ones_col = aspool.tile([128, 1], bf16, tag="ones")
            nc.vector.memset(ones_col[:], 1.0)
            rs48 = aspool.tile([1, BT], f32, tag="rs48")
            ident1 = aspool.tile([1, 1], f32, tag="ident1")
            nc.vector.memset(ident1[:], 1.0)
            for h in range(2):
                attps = apsum.tile([1, R1 // 2], f32, tag="attp")
                for mi in range(NT):
                    nc.tensor.matmul(attps[:], ones_col[:],
                                     xT_mi(mi)[:, h * 384:(h + 1) * 384],
                                     start=(mi == 0), stop=(mi == NT - 1))
                nc.vector.reduce_sum(rs48[:, h * 24:(h + 1) * 24],
                                     attps[:].rearrange("p (a b) -> p a b", b=F),
                                     axis=AX.X)
            t48ps = apsum.tile([BT, 1], f32, tag="attp")
            nc.tensor.transpose(t48ps[:], rs48[:], ident1[:])
            att0sb = aspool.tile([BT, 1], bf16, tag="att0")
            nc.scalar.activation(att0sb[:], t48ps[:], AF.Copy, scale=1.0 / (F * N))
            atbps = apsum.tile([12, 4], f32, tag="attp")
            for b in range(4):
                nc.tensor.matmul(atbps[:, b:b + 1],
                                 identb_sb[:48, b * 12:(b + 1) * 12],
                                 att0sb[:], start=True, stop=True)
            atb13 = aspool.tile([13, 4], bf16, tag="atb13")
            nc.vector.memset(atb13[:], 1.0)
            nc.scalar.activation(atb13[:12, :], atbps[:], AF.Copy)
            w1aug_sb = aspool.tile([13, 3], bf16, tag="w1aug")
            w2aug_sb = aspool.tile([4, 12], bf16, tag="w2aug")
            selE1_sb = aspool.tile([12, 12 * 128], bf16, tag="selE1")
            nc.scalar.dma_start(w1aug_sb[:], w1aug_d)
            nc.scalar.dma_start(w2aug_sb[:], w2aug_d)
            nc.scalar.dma_start(selE1_sb[:], selE1_d)
            a1ps = apsum.tile([3, 4], f32, tag="attp")
            nc.tensor.matmul(a1ps[:], w1aug_sb[:], atb13[:], start=True, stop=True)
            a1sb = aspool.tile([4, 4], bf16, tag="a1")
            nc.vector.memset(a1sb[:], 1.0)
            nc.scalar.activation(a1sb[:3, :], a1ps[:], AF.Relu)
            attps2 = apsum.tile([12, 4], f32, tag="attp")
            nc.tensor.matmul(attps2[:], w2aug_sb[:], a1sb[:], start=True, stop=True)
            att_tb = aspool.tile([12, 4], bf16, tag="att_tb")
            nc.scalar.activation(att_tb[:], attps2[:], AF.Sigmoid)
            for j in range(J1):
                b0, b1 = B01[j]
                acps = apsum.tile([128, 1], f32, tag="attp")
                nc.tensor.matmul(acps[:], selE1_sb[:, (j * 2) * 128:(j * 2 + 1) * 128],
                                 att_tb[:, b0:b0 + 1], start=True, stop=False)
                nc.tensor.matmul(acps[:], selE1_sb[:, (j * 2 + 1) * 128:(j * 2 + 2) * 128],
                                 att_tb[:, b1:b1 + 1], start=False, stop=True)
                nc.scalar.activation(attc1[:, j:j + 1], acps[:], AF.Copy)
                nc.vector.tensor_scalar_mul(th1s_sb[:, j, :], th1_sb[:],
                                            attc1[:, j:j + 1])

            # ---- gconv1: graph + feature per n-chunk ----
            chq = [ch0, ch1]
            for nci in range(NNC):
                ncs = nci * NCH
                ch = chq[nci] if nci < 2 else load_ch(nci, f"ch_{nci}")
                z0s = []
                for j1 in range(J1):
                    z0t = g1pool.tile([128, NCH], f32r, tag="z0", bufs=7,
                                      name=f"z0_{nci}_{j1}")
                    dma(z0t[:], xb_d[j1 * 128:(j1 + 1) * 128, ncs:ncs + NCH].bitcast(f32r))
                    z0s.append(z0t)

                def g1_feat(j1, zs):
                    z0t, z1t, z2t = zs
                    for ntl in range(NTL):
                        fps = fpsum.tile([128, 256], f32, tag="fps")
                        nc.tensor.matmul(fps[:],
                                         z0t[:, ntl * 128:(ntl + 1) * 128],
                                         th1s_sb[:, j1, 0:256],
                                         start=True, stop=False)
                        nc.tensor.matmul(fps[:],
                                         z1t[:, ntl * 128:(ntl + 1) * 128],
                                         th1s_sb[:, j1, 256:512],
                                         start=False, stop=False)
                        nc.tensor.matmul(fps[:],
                                         z2t[:, ntl * 128:(ntl + 1) * 128],
                                         th1s_sb[:, j1, 512:768],
                                         start=False, stop=True)
                        nc.scalar.activation(
                            xg1T_sb[:, nci * NTL + ntl, j1 * 256:(j1 + 1) * 256],
                            fps[:], AF.Relu)

                prev = None
                for j1 in range(J1):
                    zcur = [z0s[j1]]
                    ks = (1,) if t2diag else (1, 2)
                    for k in ks:
                        zps = zpsum.tile([128, NCH], f32)
                        for mi in range(NT):
                            nc.tensor.matmul(zps[:],
                                             xT_mi(mi)[:, j1 * 128:(j1 + 1) * 128],
                                             ch_k(ch, k, mi),
                                             start=(mi == 0), stop=(mi == NT - 1))
                        zt = g1pool.tile([128, NCH], f32r, tag=f"z{k}", bufs=3)
                        if k == 1:
                            nc.vector.tensor_copy(zt[:], zps[:])
                        else:
                            nc.scalar.activation(zt[:], zps[:], AF.Copy)
                        zcur.append(zt)
                    if t2diag:
                        z2t = g1pool.tile([128, NCH], f32r, tag="z2", bufs=3)
                        nc.vector.tensor_mul(z2t[:], z0s[j1][:].bitcast(f32),
                                             t2bc_sb[:, ncs:ncs + NCH])
                        zcur.append(z2t)
                    if prev is not None:
                        g1_feat(j1 - 1, prev)
                    prev = zcur
                g1_feat(J1 - 1, prev)

        if DEBUG_TAPS:
            dma(dbg_att_d, attc1[:])
            dma(dbg_xg1_d.rearrange("p (mi r) -> p mi r", r=R2), xg1T_sb[:])

        # ---------- phase C: gconv2 + temporal tail + LayerNorm ----------
        with tc.tile_pool(name="stat", bufs=1) as stpool, \
             tc.tile_pool(name="g2sb", bufs=2) as g2pool, \
             tc.tile_pool(name="tiny", bufs=8) as tinypool, \
             tc.tile_pool(name="zps2", bufs=2, space="PSUM") as zpsum2, \
             tc.tile_pool(name="tps", bufs=2, space="PSUM") as tpsum, \
             tc.tile_pool(name="fps2", bufs=2, space="PSUM") as fpsum2, \
             tc.tile_pool(name="tailps", bufs=2, space="PSUM") as tailpsum:
            stat6 = stpool.tile([128, J2, NNC, 6], f32)
            stat2 = stpool.tile([128, J2, 2], f32)
            for nci in range(NNC):
                ncs = nci * NCH
                ch = load_ch(nci, f"ch2_{nci}")
                for b in range(BC):
                    xg2t = []
                    z1l = []
                    z2l = []
                    rhl = []
                    for jt in range(3):
                        j2 = 3 * b + jt
                        zps = zpsum2.tile([128, NCH], f32, tag="zg2")
                        for mi in range(NT):
                            nc.tensor.matmul(zps[:],
                                             xg1T_sb[:, mi, j2 * 128:(j2 + 1) * 128],
                                             ch_k(ch, 1, mi),
                                             start=(mi == 0), stop=(mi == NT - 1))
                        z1t = g2pool.tile([128, NCH], f32r, tag="z1", bufs=3)
                        nc.vector.tensor_copy(z1t[:], zps[:])
                        z1l.append(z1t)
                        xg1rhs = g2pool.tile([128, NCH], f32r, tag="xg1rhs", bufs=3)
                        tp = tpsum.tile([128, NTL, 128], bf16)
                        for ntl in range(NTL):
                            nc.tensor.transpose(
                                tp[:, ntl, :], xg1T_sb[:, nci * NTL + ntl,
                                                       j2 * 128:(j2 + 1) * 128],
                                identb_sb[:])
                        nc.vector.tensor_copy(
                            xg1rhs[:].rearrange("p (a c) -> p a c", c=128), tp[:])
                        rhl.append(xg1rhs)
                        z2t = g2pool.tile([128, NCH], f32r, tag="z2", bufs=3)
                        if t2diag:
                            nc.vector.tensor_mul(z2t[:], xg1rhs[:].bitcast(f32),
                                                 t2bc_sb[:, ncs:ncs + NCH])
                        else:
                            zps2 = zpsum2.tile([128, NCH], f32, tag="zg2")
                            for mi in range(NT):
                                nc.tensor.matmul(
                                    zps2[:],
                                    xg1T_sb[:, mi, j2 * 128:(j2 + 1) * 128],
                                    ch[:, 1, mi, :],
                                    start=(mi == 0), stop=(mi == NT - 1))
                            nc.scalar.activation(z2t[:], zps2[:], AF.Copy)
                        z2l.append(z2t)
                    for jt in range(3):
                        fps = fpsum2.tile([128, NCH], f32)
                        nc.tensor.matmul(fps[:], th2_sb[:, 0:128], rhl[jt][:],
                                         start=True, stop=False)
                        nc.tensor.matmul(fps[:], th2_sb[:, 128:256], z1l[jt][:],
                                         start=False, stop=False)
                        nc.tensor.matmul(fps[:], th2_sb[:, 256:384], z2l[jt][:],
                                         start=False, stop=True)
                        xt = g2pool.tile([128, NCH], f32r, tag="xg2t", bufs=4)
                        nc.scalar.activation(xt[:], fps[:], AF.Relu)
                        xg2t.append(xt)
                        if DEBUG_TAPS:
                            j2_ = 3 * b + jt
                            dma(dbg_xg2_d[j2_ * 128:(j2_ + 1) * 128,
                                          ncs:ncs + NCH], xt[:].bitcast(f32))
                    # residual input tiles for this b (2 distinct jx)
                    xres = {}
                    for jt in range(3):
                        jx = (3 * b + jt) // 2
                        if jx not in xres:
                            xr = g2pool.tile([128, NCH], f32r, tag="xres", bufs=2,
                                             name=f"xres_{nci}_{jx}")
                            dma(xr[:], xb_d[jx * 128:(jx + 1) * 128, ncs:ncs + NCH].bitcast(f32r))
                            xres[jx] = xr
                    # temporal conv 1
                    xt1b = []
                    for jt in range(3):
                        ja, jb = TC_PAIRS[jt]
                        tp1 = tailpsum.tile([128, NCH], f32, tag="tailp")
                        ca = ((0 * 3 + jt) * 3 + ja) * 128
                        cb = ((0 * 3 + jt) * 3 + jb) * 128
                        nc.tensor.matmul(tp1[:], tcbd_sb[:, ca:ca + 128], xg2t[ja][:],
                                         start=True, stop=False)
                        nc.tensor.matmul(tp1[:], tcbd_sb[:, cb:cb + 128], xg2t[jb][:],
                                         start=False, stop=True)
                        x1 = g2pool.tile([128, NCH], f32r, tag="xt1b", bufs=4)
                        nc.scalar.activation(x1[:], tp1[:], AF.Relu,
                                             bias=bias3_sb[:, 0:1])
                        xt1b.append(x1)
                    # temporal conv 2 + residual + y chunk + stats
                    for jt in range(3):
                        j2 = 3 * b + jt
                        ja, jb = TC_PAIRS[jt]
                        tp2 = tailpsum.tile([128, NCH], f32, tag="tailp")
                        ca = ((1 * 3 + jt) * 3 + ja) * 128
                        cb = ((1 * 3 + jt) * 3 + jb) * 128
                        nc.tensor.matmul(tp2[:], tcbd_sb[:, ca:ca + 128], xt1b[ja][:],
                                         start=True, stop=False)
                        nc.tensor.matmul(tp2[:], tcbd_sb[:, cb:cb + 128], xt1b[jb][:],
                                         start=False, stop=True)
                        xt2c = g2pool.tile([128, NCH], f32r, tag="xt2c", bufs=2)
                        nc.scalar.activation(xt2c[:], tp2[:], AF.Relu,
                                             bias=bias3_sb[:, 1:2])
                        jx, half = divmod(j2, 2)
                        rp = tailpsum.tile([128, NCH], f32, tag="tailp")
                        nc.tensor.matmul(rp[:],
                                         resbd_sb[:, half * 128:(half + 1) * 128].bitcast(f32r),
                                         xres[jx][:].bitcast(f32r),
                                         start=True, stop=True)
                        if nci < NNC - 1:
                            ycur = g2pool.tile([128, NCH], f32, tag="ycur", bufs=3)
                            nc.vector.scalar_tensor_tensor(ycur[:],
                                                           rp[:], bias3_sb[:, 2:3],
                                                           xt2c[:].bitcast(f32),
                                                           ALU.add, ALU.add)
                            nc.vector.bn_stats(stat6[:, j2, nci, :], ycur[:])
                            dma(ysc_d[j2 * 128:(j2 + 1) * 128, ncs:ncs + NCH],
                                ycur[:])
                        else:
                            yrow = g2pool.tile([128, N], f32, tag="yrow", bufs=2)
                            dma(yrow[:, 0:ncs], ysc_d[j2 * 128:(j2 + 1) * 128,
                                                      0:ncs])
                            nc.vector.scalar_tensor_tensor(yrow[:, ncs:ncs + NCH],
                                                           rp[:], bias3_sb[:, 2:3],
                                                           xt2c[:].bitcast(f32),
                                                           ALU.add, ALU.add)
                            nc.vector.bn_stats(stat6[:, j2, nci, :],
                                               yrow[:, ncs:ncs + NCH])
                            nc.vector.bn_aggr(stat2[:, j2, :], stat6[:, j2, :, :])
                            varp = tinypool.tile([128, 1], f32, tag="varp")
                            nc.vector.tensor_scalar_add(varp[:],
                                                        stat2[:, j2, 1:2], 1e-5)
                            sd = tinypool.tile([128, 1], f32, tag="sd")
                            nc.scalar.sqrt(sd[:], varp[:])
                            istd = tinypool.tile([128, 1], f32, tag="istd")
                            nc.vector.reciprocal(istd[:], sd[:])
                            negmu = tinypool.tile([128, 1], f32, tag="negmu")
                            nc.vector.scalar_tensor_tensor(negmu[:],
                                                           stat2[:, j2, 0:1], -1.0,
                                                           istd[:], ALU.mult,
                                                           ALU.mult)
                            for c in range(NNC):
                                cs = c * NCH
                                yo = g2pool.tile([128, NCH], f32, tag="yout",
                                                 bufs=4)
                                if ln_affine:
                                    lgt = g2pool.tile([128, NCH], f32,
                                                      tag="lgt", bufs=2)
                                    lbt = g2pool.tile([128, NCH], f32,
                                                      tag="lbt", bufs=2)
                                    dma(lgt[:], lng_d[:, cs:cs + NCH])
                                    dma(lbt[:], lnb_d[:, cs:cs + NCH])
                                    nc.vector.tensor_scalar(yo[:],
                                                            yrow[:, cs:cs + NCH],
                                                            istd[:], negmu[:],
                                                            ALU.mult, ALU.add)
                                    nc.vector.tensor_mul(yo[:], yo[:], lgt[:])
                                    nc.vector.tensor_add(yo[:], yo[:], lbt[:])
                                    nc.scalar.activation(yo[:], yo[:], AF.Relu)
                                elif c % 2 == 0:
                                    nc.scalar.activation(yo[:],
                                                         yrow[:, cs:cs + NCH],
                                                         AF.Relu, bias=negmu[:],
                                                         scale=istd[:])
                                else:
                                    nc.vector.tensor_scalar(yo[:],
                                                            yrow[:, cs:cs + NCH],
                                                            istd[:], negmu[:],
                                                            ALU.mult, ALU.add)
                                    nc.vector.tensor_relu(yo[:], yo[:])
                                dma(y_d[j2 * 128:(j2 + 1) * 128, cs:cs + NCH],
                                    yo[:])

        chp.__exit__(None, None, None)
        xg1Tp.__exit__(None, None, None)

    nc.compile()
    return nc


def _host_prep(inputs):
    x = np.asarray(inputs["x"], np.float32)
    cheb = np.asarray(inputs["cheb"], np.float32)
    theta1 = np.asarray(inputs["theta1"], np.float32)
    theta2 = np.asarray(inputs["theta2"], np.float32)
    mlp1_w = np.asarray(inputs["mlp1_w"], np.float32)
    mlp1_b = np.asarray(inputs["mlp1_b"], np.float32)
    mlp2_w = np.asarray(inputs["mlp2_w"], np.float32)
    mlp2_b = np.asarray(inputs["mlp2_b"], np.float32)
    tc1_w = np.asarray(inputs["tc1_w"], np.float32)
    tc1_b = np.asarray(inputs["tc1_b"], np.float32)
    tc2_w = np.asarray(inputs["tc2_w"], np.float32)
    tc2_b = np.asarray(inputs["tc2_b"], np.float32)
    res_w = np.asarray(inputs["res_w"], np.float32)
    res_b = np.asarray(inputs["res_b"], np.float32)
    ln_g = np.asarray(inputs["ln_g"], np.float32)
    ln_b = np.asarray(inputs["ln_b"], np.float32)

    assert np.array_equal(cheb[0], np.eye(N, dtype=np.float32)), \
        "kernel assumes cheb[0] == I"
    ln_affine = not (np.all(ln_g == 1.0) and np.all(ln_b == 0.0))

    bfc = ml_dtypes.bfloat16
    t2 = cheb[2]
    t2d = np.diag(t2).copy()
    offmax = np.abs(t2 - np.diag(t2d)).sum(axis=1).max()
    t2diag = bool(offmax < 0.01)
    if t2diag:
        cheb_ship = {"cheb1": np.ascontiguousarray(cheb[1]).astype(bfc),
                     "t2bc": np.ascontiguousarray(
                         np.broadcast_to(t2d, (128, N))).astype(np.float32)}
    else:
        cheb_ship = {"cheb12": np.ascontiguousarray(cheb[1:3]).astype(bfc)}

    th1 = np.zeros((3, 128, 256), np.float32)
    for g in range(8):
        for k in range(3):
            th1[k, g * 16:(g + 1) * 16, g * 32:(g + 1) * 32] = theta1[k]
    th1 = np.ascontiguousarray(th1.transpose(1, 0, 2).reshape(128, 3 * 256))
    th2 = np.zeros((3, 128, 128), np.float32)
    for g in range(4):
        for k in range(3):
            th2[k, g * 32:(g + 1) * 32, g * 32:(g + 1) * 32] = theta2[k]
    th2 = np.ascontiguousarray(th2.transpose(1, 0, 2).reshape(128, 3 * 128))

    src0 = [10] + list(range(11))
    src1 = [11] + list(range(1, 12))
    tcbd = np.zeros((2, 3, 3, 128, 128), np.float32)
    for ti, w in ((0, tc1_w), (1, tc2_w)):
        for tpp in range(12):
            jt, to = divmod(tpp, 4)
            for srcs, kw in ((src0, 0), (src1, 1)):
                tin = srcs[tpp]
                ji, til = divmod(tin, 4)
                tcbd[ti, jt, ji, til * 32:(til + 1) * 32,
                     to * 32:(to + 1) * 32] += w[:, :, 0, kw].T
    tcbd = np.ascontiguousarray(
        tcbd.reshape(18, 128, 128).transpose(1, 0, 2).reshape(128, 18 * 128))

    resbd = np.zeros((2, 128, 128), np.float32)
    for half in range(2):
        for g4 in range(4):
            g = g4 + 4 * half
            resbd[half, g * 16:(g + 1) * 16,
                  g4 * 32:(g4 + 1) * 32] = res_w[:, :, 0, 0].T
    resbd = np.ascontiguousarray(
        resbd.transpose(1, 0, 2).reshape(128, 2 * 128))

    identb = np.eye(128, dtype=np.float32).astype(bfc)
    selE1 = np.zeros((6, 2, 12, 128), np.float32)
    for j in range(6):
        b0, b1 = B01[j]
        for p in range(128):
            bt = 8 * j + p // 16
            bb, t = divmod(bt, 12)
            selE1[j, 0 if bb == b0 else 1, t, p] = 1.0
    selE1 = np.ascontiguousarray(
        selE1.reshape(12, 12, 128).transpose(1, 0, 2).reshape(12, 12 * 128)
    ).astype(bfc)

    w1aug = np.concatenate([mlp1_w.T, mlp1_b[None]], 0).astype(bfc)
    w2aug = np.concatenate([mlp2_w.T, mlp2_b[None]], 0).astype(bfc)
    p32 = np.arange(128) % 32
    bias3 = np.stack([tc1_b[p32], tc2_b[p32], res_b[p32],
                      np.zeros(128, np.float32)], axis=1).astype(np.float32)

    shared = dict(cheb_ship, th1=th1, th2=th2, tcbd=tcbd, resbd=resbd,
                  identb=identb, selE1=selE1, w1aug=w1aug, w2aug=w2aug,
                  bias3=bias3)
    if ln_affine:
        shared["lng"] = np.ascontiguousarray(
            np.broadcast_to(ln_g, (128, N))).astype(np.float32)
        shared["lnb"] = np.ascontiguousarray(
            np.broadcast_to(ln_b, (128, N))).astype(np.float32)

    in_maps = []
    for c in range(NCORES):
        xc = x[c * BC:(c + 1) * BC]                       # [4, 12, 16, N]
        xT = np.ascontiguousarray(
            xc.transpose(3, 0, 1, 2).reshape(N, R1)).astype(bfc)
        xb = np.ascontiguousarray(xc.reshape(R1, N))
        in_maps.append(dict(shared, xT=xT, xb=xb))
    return in_maps, ln_affine, t2diag


def kernel(**inputs):
    global _compiled, _compiled_affine
    in_maps, ln_affine, t2diag = _host_prep(inputs)
    if _compiled is None or _compiled_affine != (ln_affine, t2diag):
        _compiled = _build(ln_affine, t2diag)
        _compiled_affine = (ln_affine, t2diag)
    res = run_bass_kernel_spmd(_compiled, in_maps, list(range(NCORES)))
    y = np.empty((B, T, O, N), np.float32)
    for c in range(NCORES):
        y[c * BC:(c + 1) * BC] = res.results[c]["y"].reshape(BC, T, O, N)
    return y


# revision 5
# speedup vs baseline: 1.5744x; 1.0082x over previous
"""Trainium2 Bass kernel for nn_Branch1_block (gnn_message_passing), v2.

Data-parallel over batch on 8 NeuronCores (4 batches/core). All matmul
operands in bf16 (PSUM accumulation fp32): halves DMA + SBUF vs fp32r at
the same PE rate. T2 Chebyshev matrix precomputed on host (no on-chip
recurrence). SE attention is folded into per-j1 scaled copies of the
theta1 block-diagonal (no per-z scaling on DVE). xg2 never leaves SBUF:
the temporal-conv tail is interleaved per n-chunk into gconv2, and
LayerNorm stats accumulate per-chunk via bn_stats/bn_aggr.
"""
import sys

import numpy as np

try:
    import concourse.bass as bass
except ImportError:  # pragma: no cover - grading env fallback
    for p in ("/root/.axon_site", "/root/.axon_site/_ro/trn_rl_repo",
              "/root/.axon_site/_ro/pypackages", "/opt/trn_rl_repo"):
        if p not in sys.path:
            sys.path.append(p)
    import concourse.bass as bass

from contextlib import ExitStack

import ml_dtypes
import concourse.mybir as mybir
import concourse.tile as tile
from concourse import bacc
from concourse.bass_utils import run_bass_kernel_spmd

B, T, F, O, N, K = 32, 12, 16, 32, 2048, 3
NCORES = 8
BC = B // NCORES          # 4 batches per core
BT = BC * T               # 48
R1 = BT * F               # 768 rows (bt,f)
R2 = BT * O               # 1536 rows (bt,o)
J1 = R1 // 128            # 6
J2 = R2 // 128            # 12
NT = N // 128             # 16
NCH = 512
NNC = N // NCH            # 4
NTL = NCH // 128          # 4 n-tiles per chunk

f32 = mybir.dt.float32
f32r = mybir.dt.float32r
bf16 = mybir.dt.bfloat16
AF = mybir.ActivationFunctionType
AX = mybir.AxisListType
ALU = mybir.AluOpType

# (jt, ji) pairs with nonzero temporal-conv block matrices
TC_PAIRS = {0: (0, 2), 1: (0, 1), 2: (1, 2)}
# per j1 (r1 tile): the one or two batches its rows touch
B01 = []
for _j in range(6):
    _bs = sorted({(8 * _j + _p // 16) // 12 for _p in range(128)})
    B01.append((_bs[0], _bs[-1]))

_compiled = None
_compiled_affine = None


DEBUG_TAPS = False


def _build(ln_affine, t2diag):
    nc = bacc.Bacc("TRN2", target_bir_lowering=False, debug=False)

    xT_d = nc.dram_tensor("xT", [N, R1], bf16, kind="ExternalInput").ap()
    xb_d = nc.dram_tensor("xb", [R1, N], f32, kind="ExternalInput").ap()
    if t2diag:
        cheb_d = nc.dram_tensor("cheb1", [N, N], bf16, kind="ExternalInput").ap()
        t2bc_d = nc.dram_tensor("t2bc", [128, N], f32, kind="ExternalInput").ap()
    else:
        cheb_d = nc.dram_tensor("cheb12", [2, N, N], bf16,
                                kind="ExternalInput").ap()
    th1_d = nc.dram_tensor("th1", [128, 3 * 256], f32, kind="ExternalInput").ap()
    th2_d = nc.dram_tensor("th2", [128, 3 * 128], f32, kind="ExternalInput").ap()
    tcbd_d = nc.dram_tensor("tcbd", [128, 18 * 128], f32, kind="ExternalInput").ap()
    resbd_d = nc.dram_tensor("resbd", [128, 2 * 128], f32, kind="ExternalInput").ap()
    identb_d = nc.dram_tensor("identb", [128, 128], bf16, kind="ExternalInput").ap()
    selE1_d = nc.dram_tensor("selE1", [12, 12 * 128], bf16, kind="ExternalInput").ap()
    w1aug_d = nc.dram_tensor("w1aug", [13, 3], bf16, kind="ExternalInput").ap()
    w2aug_d = nc.dram_tensor("w2aug", [4, 12], bf16, kind="ExternalInput").ap()
    bias3_d = nc.dram_tensor("bias3", [128, 4], f32, kind="ExternalInput").ap()
    if ln_affine:
        lng_d = nc.dram_tensor("lng", [128, N], f32, kind="ExternalInput").ap()
        lnb_d = nc.dram_tensor("lnb", [128, N], f32, kind="ExternalInput").ap()
    y_d = nc.dram_tensor("y", [R2, N], f32, kind="ExternalOutput").ap()
    ysc_d = nc.dram_tensor("ysc", [R2, N], f32).ap()
    if DEBUG_TAPS:
        dbg_att_d = nc.dram_tensor("dbg_att", [128, J1], f32,
                                   kind="ExternalOutput").ap()
        dbg_xg1_d = nc.dram_tensor("dbg_xg1", [128, NT * R2], bf16,
                                   kind="ExternalOutput").ap()
        dbg_xg2_d = nc.dram_tensor("dbg_xg2", [R2, N], f32,
                                   kind="ExternalOutput").ap()

    dma = nc.sync.dma_start

    with tile.TileContext(nc) as tc, ExitStack() as top:
        cpool = top.enter_context(tc.tile_pool(name="const", bufs=1))
        th2_sb = cpool.tile([128, 3 * 128], f32r)
        if t2diag:
            t2bc_sb = cpool.tile([128, N], f32)
        tcbd_sb = cpool.tile([128, 18 * 128], f32r)
        resbd_sb = cpool.tile([128, 2 * 128], f32r)
        identb_sb = cpool.tile([128, 128], bf16)
        bias3_sb = cpool.tile([128, 4], f32)
        attc1 = cpool.tile([128, J1], f32)
        cdma = nc.scalar.dma_start
        if t2diag:
            cdma(t2bc_sb[:], t2bc_d)
        cdma(th2_sb[:], th2_d.bitcast(f32r))
        cdma(tcbd_sb[:], tcbd_d.bitcast(f32r))
        cdma(resbd_sb[:], resbd_d.bitcast(f32r))
        cdma(identb_sb[:], identb_d)
        cdma(bias3_sb[:], bias3_d)

        xg1Tp = tc.tile_pool(name="xg1T", bufs=1)
        xg1Tpool = xg1Tp.__enter__()
        xg1T_sb = xg1Tpool.tile([128, NT, R2], bf16)

        chp = tc.tile_pool(name="chp", bufs=2)
        chpool = chp.__enter__()

        def load_ch(nci, name):
            ncs = nci * NCH
            if t2diag:
                ch = chpool.tile([128, NT, NCH], bf16, tag="ch", bufs=2,
                                 name=name)
                chv = cheb_d[:, ncs:ncs + NCH].rearrange(
                    "(mi p) n -> mi p n", p=128)
                for q in range(2):
                    dma(ch[:, q * 8:(q + 1) * 8, :],
                        chv[q * 8:(q + 1) * 8].rearrange("mi p n -> p mi n"))
                return ch
            ch = chpool.tile([128, 2, NT, NCH], bf16, tag="ch", bufs=2, name=name)
            chv = cheb_d[:, :, ncs:ncs + NCH].rearrange(
                "t (mi p) n -> t mi p n", p=128)
            for t_ in range(2):
                for q in range(2):
                    dma(ch[:, t_, q * 8:(q + 1) * 8, :],
                        chv[t_, q * 8:(q + 1) * 8].rearrange("mi p n -> p mi n"))
            return ch

        def ch_k(ch, k, mi):
            return ch[:, mi, :] if t2diag else ch[:, k - 1, mi, :]

        # ---------- phase A+B: attention + gconv1 ----------
        with tc.tile_pool(name="xTp", bufs=1) as xTpool, \
             tc.tile_pool(name="g1sb", bufs=2) as g1pool, \
             tc.tile_pool(name="attps", bufs=2, space="PSUM") as apsum, \
             tc.tile_pool(name="attsb", bufs=2) as aspool, \
             tc.tile_pool(name="zps", bufs=3, space="PSUM") as zpsum, \
             tc.tile_pool(name="fps", bufs=3, space="PSUM") as fpsum:
            xTq = [xTpool.tile([128, 4, R1], bf16, name=f"xTq{q}")
                   for q in range(4)]
            th1_sb = xTpool.tile([128, 3 * 256], f32)
            th1s_sb = xTpool.tile([128, J1, 3 * 256], f32r)
            nc.scalar.dma_start(th1_sb[:], th1_d)
            xTv = xT_d.rearrange("(mi p) r -> mi p r", p=128)
            for q in range(4):
                dma(xTq[q][:], xTv[q * 4:(q + 1) * 4].rearrange("mi p r -> p mi r"))

            def xT_mi(mi):
                return xTq[mi // 4][:, mi % 4, :]
            ch0 = load_ch(0, "ch_0")
            ch1 = load_ch(1, "ch_1")

            # ---- SE attention ----
            ones_col = aspool.tile([128, 1], bf16, tag="ones")
            nc.vector.memset(ones_col[:], 1.0)
            rs48 = aspool.tile([1, BT], f32, tag="rs48")
            ident1 = aspool.tile([1, 1], f32, tag="ident1")
            nc.vector.memset(ident1[:], 1.0)
            for h in range(2):
                attps = apsum.tile([1, R1 // 2], f32, tag="attp")
                for mi in range(NT):
                    nc.tensor.matmul(attps[:], ones_col[:],
                                     xT_mi(mi)[:, h * 384:(h + 1) * 384],
                                     start=(mi == 0), stop=(mi == NT - 1))
                nc.vector.reduce_sum(rs48[:, h * 24:(h + 1) * 24],
                                     attps[:].rearrange("p (a b) -> p a b", b=F),
                                     axis=AX.X)
            t48ps = apsum.tile([BT, 1], f32, tag="attp")
            nc.tensor.transpose(t48ps[:], rs48[:], ident1[:])
            att0sb = aspool.tile([BT, 1], bf16, tag="att0")
            nc.scalar.activation(att0sb[:], t48ps[:], AF.Copy, scale=1.0 / (F * N))
            atbps = apsum.tile([12, 4], f32, tag="attp")
            for b in range(4):
                nc.tensor.matmul(atbps[:, b:b + 1],
                                 identb_sb[:48, b * 12:(b + 1) * 12],
                                 att0sb[:], start=True, stop=True)
            atb13 = aspool.tile([13, 4], bf16, tag="atb13")
            nc.vector.memset(atb13[:], 1.0)
            nc.scalar.activation(atb13[:12, :], atbps[:], AF.Copy)
            w1aug_sb = aspool.tile([13, 3], bf16, tag="w1aug")
            w2aug_sb = aspool.tile([4, 12], bf16, tag="w2aug")
            selE1_sb = aspool.tile([12, 12 * 128], bf16, tag="selE1")
            nc.scalar.dma_start(w1aug_sb[:], w1aug_d)
            nc.scalar.dma_start(w2aug_sb[:], w2aug_d)
            nc.scalar.dma_start(selE1_sb[:], selE1_d)
            a1ps = apsum.tile([3, 4], f32, tag="attp")
            nc.tensor.matmul(a1ps[:], w1aug_sb[:], atb13[:], start=True, stop=True)
            a1sb = aspool.tile([4, 4], bf16, tag="a1")
            nc.vector.memset(a1sb[:], 1.0)
            nc.scalar.activation(a1sb[:3, :], a1ps[:], AF.Relu)
            attps2 = apsum.tile([12, 4], f32, tag="attp")
            nc.tensor.matmul(attps2[:], w2aug_sb[:], a1sb[:], start=True, stop=True)
            att_tb = aspool.tile([12, 4], bf16, tag="att_tb")
            nc.scalar.activation(att_tb[:], attps2[:], AF.Sigmoid)
            for j in range(J1):
                b0, b1 = B01[j]
                acps = apsum.tile([128, 1], f32, tag="attp")
                nc.tensor.matmul(acps[:], selE1_sb[:, (j * 2) * 128:(j * 2 + 1) * 128],
                                 att_tb[:, b0:b0 + 1], start=True, stop=False)
                nc.tensor.matmul(acps[:], selE1_sb[:, (j * 2 + 1) * 128:(j * 2 + 2) * 128],
                                 att_tb[:, b1:b1 + 1], start=False, stop=True)
                nc.scalar.activation(attc1[:, j:j + 1], acps[:], AF.Copy)
                nc.vector.tensor_scalar_mul(th1s_sb[:, j, :], th1_sb[:],
                                            attc1[:, j:j + 1])

            # ---- gconv1: graph + feature per n-chunk ----
            chq = [ch0, ch1]
            for nci in range(NNC):
                ncs = nci * NCH
                ch = chq[nci] if nci < 2 else load_ch(nci, f"ch_{nci}")
                z0s = []
                for j1 in range(J1):
                    z0t = g1pool.tile([128, NCH], f32r, tag="z0", bufs=7,
                                      name=f"z0_{nci}_{j1}")
                    dma(z0t[:], xb_d[j1 * 128:(j1 + 1) * 128, ncs:ncs + NCH].bitcast(f32r))
                    z0s.append(z0t)

                def g1_feat(j1, zs):
                    z0t, z1t, z2t = zs
                    for ntl in range(NTL):
                        fps = fpsum.tile([128, 256], f32, tag="fps")
                        nc.tensor.matmul(fps[:],
                                         z0t[:, ntl * 128:(ntl + 1) * 128],
                                         th1s_sb[:, j1, 0:256],
                                         start=True, stop=False)
                        nc.tensor.matmul(fps[:],
                                         z1t[:, ntl * 128:(ntl + 1) * 128],
                                         th1s_sb[:, j1, 256:512],
                                         start=False, stop=False)
                        nc.tensor.matmul(fps[:],
                                         z2t[:, ntl * 128:(ntl + 1) * 128],
                                         th1s_sb[:, j1, 512:768],
                                         start=False, stop=True)
                        nc.scalar.activation(
                            xg1T_sb[:, nci * NTL + ntl, j1 * 256:(j1 + 1) * 256],
                            fps[:], AF.Relu)

                prev = None
                for j1 in range(J1):
                    zcur = [z0s[j1]]
                    ks = (1,) if t2diag else (1, 2)
                    for k in ks:
                        zps = zpsum.tile([128, NCH], f32)
                        for mi in range(NT):
                            nc.tensor.matmul(zps[:],
                                             xT_mi(mi)[:, j1 * 128:(j1 + 1) * 128],
                                             ch_k(ch, k, mi),
                                             start=(mi == 0), stop=(mi == NT - 1))
                        zt = g1pool.tile([128, NCH], f32r, tag=f"z{k}", bufs=3)
                        if k == 1:
                            nc.vector.tensor_copy(zt[:], zps[:])
                        else:
                            nc.scalar.activation(zt[:], zps[:], AF.Copy)
                        zcur.append(zt)
                    if t2diag:
                        z2t = g1pool.tile([128, NCH], f32r, tag="z2", bufs=3)
                        nc.vector.tensor_mul(z2t[:], z0s[j1][:].bitcast(f32),
                                             t2bc_sb[:, ncs:ncs + NCH])
                        zcur.append(z2t)
                    if prev is not None:
                        g1_feat(j1 - 1, prev)
                    prev = zcur
                g1_feat(J1 - 1, prev)

        if DEBUG_TAPS:
            dma(dbg_att_d, attc1[:])
            dma(dbg_xg1_d.rearrange("p (mi r) -> p mi r", r=R2), xg1T_sb[:])

        # ---------- phase C: gconv2 + temporal tail + LayerNorm ----------
        with tc.tile_pool(name="stat", bufs=1) as stpool, \
             tc.tile_pool(name="g2sb", bufs=2) as g2pool, \
             tc.tile_pool(name="tiny", bufs=8) as tinypool, \
             tc.tile_pool(name="zps2", bufs=2, space="PSUM") as zpsum2, \
             tc.tile_pool(name="tps", bufs=2, space="PSUM") as tpsum, \
             tc.tile_pool(name="fps2", bufs=2, space="PSUM") as fpsum2, \
             tc.tile_pool(name="tailps", bufs=2, space="PSUM") as tailpsum:
            stat6 = stpool.tile([128, J2, NNC, 6], f32)
            stat2 = stpool.tile([128, J2, 2], f32)
            for nci in range(NNC):
                ncs = nci * NCH
                ch = load_ch(nci, f"ch2_{nci}")
                for b in range(BC):
                    xg2t = []
                    z1l = []
                    z2l = []
                    rhl = []
                    for jt in range(3):
                        j2 = 3 * b + jt
                        zps = zpsum2.tile([128, NCH], f32, tag="zg2")
                        for mi in range(NT):
                            nc.tensor.matmul(zps[:],
                                             xg1T_sb[:, mi, j2 * 128:(j2 + 1) * 128],
                                             ch_k(ch, 1, mi),
                                             start=(mi == 0), stop=(mi == NT - 1))
                        z1t = g2pool.tile([128, NCH], f32r, tag="z1", bufs=3)
                        nc.vector.tensor_copy(z1t[:], zps[:])
                        z1l.append(z1t)
                        xg1rhs = g2pool.tile([128, NCH], f32r, tag="xg1rhs", bufs=3)
                        tp = tpsum.tile([128, NTL, 128], bf16)
                        for ntl in range(NTL):
                            nc.tensor.transpose(
                                tp[:, ntl, :], xg1T_sb[:, nci * NTL + ntl,
                                                       j2 * 128:(j2 + 1) * 128],
                                identb_sb[:])
                        nc.vector.tensor_copy(
                            xg1rhs[:].rearrange("p (a c) -> p a c", c=128), tp[:])
                        rhl.append(xg1rhs)
                        z2t = g2pool.tile([128, NCH], f32r, tag="z2", bufs=3)
                        if t2diag:
                            nc.vector.tensor_mul(z2t[:], xg1rhs[:].bitcast(f32),
                                                 t2bc_sb[:, ncs:ncs + NCH])
                        else:
                            zps2 = zpsum2.tile([128, NCH], f32, tag="zg2")
                            for mi in range(NT):
                                nc.tensor.matmul(
                                    zps2[:],
                                    xg1T_sb[:, mi, j2 * 128:(j2 + 1) * 128],
                                    ch[:, 1, mi, :],
                                    start=(mi == 0), stop=(mi == NT - 1))
                            nc.scalar.activation(z2t[:], zps2[:], AF.Copy)
                        z2l.append(z2t)
                    for jt in range(3):
                        fps = fpsum2.tile([128, NCH], f32)
                        nc.tensor.matmul(fps[:], th2_sb[:, 0:128], rhl[jt][:],
                                         start=True, stop=False)
                        nc.tensor.matmul(fps[:], th2_sb[:, 128:256], z1l[jt][:],
                                         start=False, stop=False)
                        nc.tensor.matmul(fps[:], th2_sb[:, 256:384], z2l[jt][:],
                                         start=False, stop=True)
                        xt = g2pool.tile([128, NCH], f32r, tag="xg2t", bufs=4)
                        nc.scalar.activation(xt[:], fps[:], AF.Relu)
                        xg2t.append(xt)
                        if DEBUG_TAPS:
                            j2_ = 3 * b + jt
                            dma(dbg_xg2_d[j2_ * 128:(j2_ + 1) * 128,
                                          ncs:ncs + NCH], xt[:].bitcast(f32))
                    # residual input tiles for this b (2 distinct jx)
                    xres = {}
                    for jt in range(3):
                        jx = (3 * b + jt) // 2
                        if jx not in xres:
                            xr = g2pool.tile([128, NCH], f32r, tag="xres", bufs=2,
                                             name=f"xres_{nci}_{jx}")
                            xr_issue = nc.scalar.dma_start if nci == NNC - 1 else dma
                            xr_issue(xr[:], xb_d[jx * 128:(jx + 1) * 128,
                                                 ncs:ncs + NCH].bitcast(f32r))
                            xres[jx] = xr
                    # temporal conv 1
                    xt1b = []
                    for jt in range(3):
                        ja, jb = TC_PAIRS[jt]
                        tp1 = tailpsum.tile([128, NCH], f32, tag="tailp")
                        ca = ((0 * 3 + jt) * 3 + ja) * 128
                        cb = ((0 * 3 + jt) * 3 + jb) * 128
                        nc.tensor.matmul(tp1[:], tcbd_sb[:, ca:ca + 128], xg2t[ja][:],
                                         start=True, stop=False)
                        nc.tensor.matmul(tp1[:], tcbd_sb[:, cb:cb + 128], xg2t[jb][:],
                                         start=False, stop=True)
                        x1 = g2pool.tile([128, NCH], f32r, tag="xt1b", bufs=4)
                        nc.scalar.activation(x1[:], tp1[:], AF.Relu,
                                             bias=bias3_sb[:, 0:1])
                        xt1b.append(x1)
                    # temporal conv 2 + residual + y chunk + stats
                    for jt in range(3):
                        j2 = 3 * b + jt
                        ja, jb = TC_PAIRS[jt]
                        tp2 = tailpsum.tile([128, NCH], f32, tag="tailp")
                        ca = ((1 * 3 + jt) * 3 + ja) * 128
                        cb = ((1 * 3 + jt) * 3 + jb) * 128
                        nc.tensor.matmul(tp2[:], tcbd_sb[:, ca:ca + 128], xt1b[ja][:],
                                         start=True, stop=False)
                        nc.tensor.matmul(tp2[:], tcbd_sb[:, cb:cb + 128], xt1b[jb][:],
                                         start=False, stop=True)
                        xt2c = g2pool.tile([128, NCH], f32r, tag="xt2c", bufs=2)
                        nc.scalar.activation(xt2c[:], tp2[:], AF.Relu,
                                             bias=bias3_sb[:, 1:2])
                        jx, half = divmod(j2, 2)
                        rp = tailpsum.tile([128, NCH], f32, tag="tailp")
                        nc.tensor.matmul(rp[:],
                                         resbd_sb[:, half * 128:(half + 1) * 128].bitcast(f32r),
                                         xres[jx][:].bitcast(f32r),
                                         start=True, stop=True)
                        if nci < NNC - 1:
                            ycur = g2pool.tile([128, NCH], f32, tag="ycur", bufs=3)
                            nc.vector.scalar_tensor_tensor(ycur[:],
                                                           rp[:], bias3_sb[:, 2:3],
                                                           xt2c[:].bitcast(f32),
                                                           ALU.add, ALU.add)
                            nc.vector.bn_stats(stat6[:, j2, nci, :], ycur[:])
                            dma(ysc_d[j2 * 128:(j2 + 1) * 128, ncs:ncs + NCH],
                                ycur[:])
                        else:
                            yrow = g2pool.tile([128, N], f32, tag="yrow", bufs=2)
                            dma(yrow[:, 0:ncs], ysc_d[j2 * 128:(j2 + 1) * 128,
                                                      0:ncs])
                            nc.vector.scalar_tensor_tensor(yrow[:, ncs:ncs + NCH],
                                                           rp[:], bias3_sb[:, 2:3],
                                                           xt2c[:].bitcast(f32),
                                                           ALU.add, ALU.add)
                            nc.vector.bn_stats(stat6[:, j2, nci, :],
                                               yrow[:, ncs:ncs + NCH])
                            nc.vector.bn_aggr(stat2[:, j2, :], stat6[:, j2, :, :])
                            varp = tinypool.tile([128, 1], f32, tag="varp")
                            nc.vector.tensor_scalar_add(varp[:],
                                                        stat2[:, j2, 1:2], 1e-5)
                            sd = tinypool.tile([128, 1], f32, tag="sd")
                            nc.scalar.sqrt(sd[:], varp[:])
                            istd = tinypool.tile([128, 1], f32, tag="istd")
                            nc.vector.reciprocal(istd[:], sd[:])
                            negmu = tinypool.tile([128, 1], f32, tag="negmu")
                            nc.vector.scalar_tensor_tensor(negmu[:],
                                                           stat2[:, j2, 0:1], -1.0,
                                                           istd[:], ALU.mult,
                                                           ALU.mult)
                            for c in range(NNC):
                                cs = c * NCH
                                yo = g2pool.tile([128, NCH], f32, tag="yout",
                                                 bufs=4)
                                if ln_affine:
                                    lgt = g2pool.tile([128, NCH], f32,
                                                      tag="lgt", bufs=2)
                                    lbt = g2pool.tile([128, NCH], f32,
                                                      tag="lbt", bufs=2)
                                    dma(lgt[:], lng_d[:, cs:cs + NCH])
                                    dma(lbt[:], lnb_d[:, cs:cs + NCH])
                                    nc.vector.tensor_scalar(yo[:],
                                                            yrow[:, cs:cs + NCH],
                                                            istd[:], negmu[:],
                                                            ALU.mult, ALU.add)
                                    nc.vector.tensor_mul(yo[:], yo[:], lgt[:])
                                    nc.vector.tensor_add(yo[:], yo[:], lbt[:])
                                    nc.scalar.activation(yo[:], yo[:], AF.Relu)
                                elif c % 2 == 0:
                                    nc.scalar.activation(yo[:],
                                                         yrow[:, cs:cs + NCH],
                                                         AF.Relu, bias=negmu[:],
                                                         scale=istd[:])
                                else:
                                    nc.vector.tensor_scalar(yo[:],
                                                            yrow[:, cs:cs + NCH],
                                                            istd[:], negmu[:],
                                                            ALU.mult, ALU.add)
                                    nc.vector.tensor_relu(yo[:], yo[:])
                                dma(y_d[j2 * 128:(j2 + 1) * 128, cs:cs + NCH],
                                    yo[:])

        chp.__exit__(None, None, None)
        xg1Tp.__exit__(None, None, None)

    nc.compile()
    return nc


def _host_prep(inputs):
    x = np.asarray(inputs["x"], np.float32)
    cheb = np.asarray(inputs["cheb"], np.float32)
    theta1 = np.asarray(inputs["theta1"], np.float32)
    theta2 = np.asarray(inputs["theta2"], np.float32)
    mlp1_w = np.asarray(inputs["mlp1_w"], np.float32)
    mlp1_b = np.asarray(inputs["mlp1_b"], np.float32)
    mlp2_w = np.asarray(inputs["mlp2_w"], np.float32)
    mlp2_b = np.asarray(inputs["mlp2_b"], np.float32)
    tc1_w = np.asarray(inputs["tc1_w"], np.float32)
    tc1_b = np.asarray(inputs["tc1_b"], np.float32)
    tc2_w = np.asarray(inputs["tc2_w"], np.float32)
    tc2_b = np.asarray(inputs["tc2_b"], np.float32)
    res_w = np.asarray(inputs["res_w"], np.float32)
    res_b = np.asarray(inputs["res_b"], np.float32)
    ln_g = np.asarray(inputs["ln_g"], np.float32)
    ln_b = np.asarray(inputs["ln_b"], np.float32)

    assert np.array_equal(cheb[0], np.eye(N, dtype=np.float32)), \
        "kernel assumes cheb[0] == I"
    ln_affine = not (np.all(ln_g == 1.0) and np.all(ln_b == 0.0))

    bfc = ml_dtypes.bfloat16
    t2 = cheb[2]
    t2d = np.diag(t2).copy()
    offmax = np.abs(t2 - np.diag(t2d)).sum(axis=1).max()
    t2diag = bool(offmax < 0.01)
    if t2diag:
        cheb_ship = {"cheb1": np.ascontiguousarray(cheb[1]).astype(bfc),
                     "t2bc": np.ascontiguousarray(
                         np.broadcast_to(t2d, (128, N))).astype(np.float32)}
    else:
        cheb_ship = {"cheb12": np.ascontiguousarray(cheb[1:3]).astype(bfc)}

    th1 = np.zeros((3, 128, 256), np.float32)
    for g in range(8):
        for k in range(3):
            th1[k, g * 16:(g + 1) * 16, g * 32:(g + 1) * 32] = theta1[k]
    th1 = np.ascontiguousarray(th1.transpose(1, 0, 2).reshape(128, 3 * 256))
    th2 = np.zeros((3, 128, 128), np.float32)
    for g in range(4):
        for k in range(3):
            th2[k, g * 32:(g + 1) * 32, g * 32:(g + 1) * 32] = theta2[k]
    th2 = np.ascontiguousarray(th2.transpose(1, 0, 2).reshape(128, 3 * 128))

    src0 = [10] + list(range(11))
    src1 = [11] + list(range(1, 12))
    tcbd = np.zeros((2, 3, 3, 128, 128), np.float32)
    for ti, w in ((0, tc1_w), (1, tc2_w)):
        for tpp in range(12):
            jt, to = divmod(tpp, 4)
            for srcs, kw in ((src0, 0), (src1, 1)):
                tin = srcs[tpp]
                ji, til = divmod(tin, 4)
                tcbd[ti, jt, ji, til * 32:(til + 1) * 32,
                     to * 32:(to + 1) * 32] += w[:, :, 0, kw].T
    tcbd = np.ascontiguousarray(
        tcbd.reshape(18, 128, 128).transpose(1, 0, 2).reshape(128, 18 * 128))

    resbd = np.zeros((2, 128, 128), np.float32)
    for half in range(2):
        for g4 in range(4):
            g = g4 + 4 * half
            resbd[half, g * 16:(g + 1) * 16,
                  g4 * 32:(g4 + 1) * 32] = res_w[:, :, 0, 0].T
    resbd = np.ascontiguousarray(
        resbd.transpose(1, 0, 2).reshape(128, 2 * 128))

    identb = np.eye(128, dtype=np.float32).astype(bfc)
    selE1 = np.zeros((6, 2, 12, 128), np.float32)
    for j in range(6):
        b0, b1 = B01[j]
        for p in range(128):
            bt = 8 * j + p // 16
            bb, t = divmod(bt, 12)
            selE1[j, 0 if bb == b0 else 1, t, p] = 1.0
    selE1 = np.ascontiguousarray(
        selE1.reshape(12, 12, 128).transpose(1, 0, 2).reshape(12, 12 * 128)
    ).astype(bfc)

    w1aug = np.concatenate([mlp1_w.T, mlp1_b[None]], 0).astype(bfc)
    w2aug = np.concatenate([mlp2_w.T, mlp2_b[None]], 0).astype(bfc)
    p32 = np.arange(128) % 32
    bias3 = np.stack([tc1_b[p32], tc2_b[p32], res_b[p32],
                      np.zeros(128, np.float32)], axis=1).astype(np.float32)

    shared = dict(cheb_ship, th1=th1, th2=th2, tcbd=tcbd, resbd=resbd,
                  identb=identb, selE1=selE1, w1aug=w1aug, w2aug=w2aug,
                  bias3=bias3)
    if ln_affine:
        shared["lng"] = np.ascontiguousarray(
            np.broadcast_to(ln_g, (128, N))).astype(np.float32)
        shared["lnb"] = np.ascontiguousarray(
            np.broadcast_to(ln_b, (128, N))).astype(np.float32)

    in_maps = []
    for c in range(NCORES):
        xc = x[c * BC:(c + 1) * BC]                       # [4, 12, 16, N]
        xT = np.ascontiguousarray(
            xc.transpose(3, 0, 1, 2).reshape(N, R1)).astype(bfc)
        xb = np.ascontiguousarray(xc.reshape(R1, N))
        in_maps.append(dict(shared, xT=xT, xb=xb))
    return in_maps, ln_affine, t2diag


def kernel(**inputs):
    global _compiled, _compiled_affine
    in_maps, ln_affine, t2diag = _host_prep(inputs)
    if _compiled is None or _compiled_affine != (ln_affine, t2diag):
        _compiled = _build(ln_affine, t2diag)
        _compiled_affine = (ln_affine, t2diag)
    res = run_bass_kernel_spmd(_compiled, in_maps, list(range(NCORES)))
    y = np.empty((B, T, O, N), np.float32)
    for c in range(NCORES):
        y[c * BC:(c + 1) * BC] = res.results[c]["y"].reshape(BC, T, O, N)
    return y
